# revision 1
# baseline (speedup 1.0000x reference)
"""FeatureProcessingBlock Trainium kernel.

out = sum_t einsum('bcphqw,twW,thH,tcC->bCpHqW', x.reshape(B,C,P,64,Q,64), Ws, Hs, Cs)

Sharding: 8 cores = (B=4) x (H-halves=2); each core gets x[b, :, ph*256:(ph+1)*256, :]
i.e. a [C=48, 256, 512] slab = 4 p-blocks x 8 q-blocks of 64x64 windows.

Per-core pipeline (single pass over HBM, f32r matmuls):
  h-stage:  tiles [h | (c,w)] -> Y = Hs_t^T X        (K=64 matmul, H' on psum partitions)
  swap1:    DVE stream-transpose (32x32 blocks) PSUM->SBUF: w-sub on partitions
  w-stage:  K=32 matmuls (row-groups by H'-half), accumulate over w-halves
  swap2:    stream-transpose PSUM->SBUF: c on partitions
  c-stage:  K=32 matmuls (row-groups by W'-half), accumulate over t and c-halves
            (c 48..63 are duplicated junk lanes killed by zero rows in the Cs tile)
  exit:     copy psum->SBUF, DMA out
"""

import numpy as np

B, C, H, W = 4, 48, 512, 512
T, WS = 3, 64
NCORES = 8
PH = H // 2  # rows per core

LAST_EXEC_NS = None

_CACHE = {}


def _build(np_blocks, nq_blocks):
    """Build the bass module for a shard of [C, np_blocks*64, nq_blocks*64]."""
    import concourse.bacc as bacc
    import concourse.mybir as mybir
    from concourse.bass import MemorySpace
    from concourse.tile import TileContext

    F32 = mybir.dt.float32
    F32R = mybir.dt.float32r

    HS_ROWS = np_blocks * 64
    WS_COLS = nq_blocks * 64

    nc = bacc.Bacc("TRN2", target_bir_lowering=False, debug=False, num_devices=NCORES)
    x = nc.dram_tensor("x", [C, HS_ROWS, WS_COLS], F32R, kind="ExternalInput")
    hs_d = nc.dram_tensor("hs", [T, 64, 64], F32R, kind="ExternalInput")
    ws_d = nc.dram_tensor("ws", [T, 64, 64], F32R, kind="ExternalInput")
    cs_d = nc.dram_tensor("cs", [T, C, C], F32R, kind="ExternalInput")
    out = nc.dram_tensor("out", [C, HS_ROWS, WS_COLS], F32, kind="ExternalOutput")

    with TileContext(nc) as tc:
        with (
            tc.tile_pool(name="consts", bufs=1) as consts,
            tc.tile_pool(name="xin", bufs=4) as xin,
            tc.tile_pool(name="vbuf", bufs=3) as vbuf,
            tc.tile_pool(name="ubuf", bufs=3) as ubuf,
            tc.tile_pool(name="obuf", bufs=3) as obuf,
            tc.tile_pool(name="ypsum", bufs=2, space=MemorySpace.PSUM) as ypsum,
            tc.tile_pool(name="zpsum", bufs=4, space=MemorySpace.PSUM) as zpsum,
            tc.tile_pool(name="opsum", bufs=1, space=MemorySpace.PSUM) as opsum,
        ):
            # ---- constant factor tiles ----
            # Hs: [h | (t, H')]
            hs_sb = consts.tile([64, T, 64], F32R)
            nc.sync.dma_start(out=hs_sb, in_=hs_d[:, :, :].rearrange("t h H -> h t H"))
            # Ws replicated on all 4 partition groups: [32g + wsub | (t, wb, W')]
            ws_sb = consts.tile([128, T, 2, 64], F32R)
            for g in range(4):
                nc.sync.dma_start(
                    out=ws_sb[32 * g : 32 * g + 32],
                    in_=ws_d[:, :, :].rearrange("t (wb u) W -> u t wb W", wb=2),
                )
            # Cs stream-A rows c0..31, replicated on groups 0-1
            csa_sb = consts.tile([64, T, C], F32R)
            for g in range(2):
                nc.sync.dma_start(
                    out=csa_sb[32 * g : 32 * g + 32],
                    in_=cs_d[:, 0:32, :].rearrange("t c C -> c t C"),
                )
            # Cs stream-B: rows 0-15 = Cs[t, 32:48], rows 16-31 zero
            csb_sb = consts.tile([64, T, C], F32R)
            nc.any.memzero(csb_sb)
            for g in range(2):
                nc.sync.dma_start(
                    out=csb_sb[32 * g : 32 * g + 16],
                    in_=cs_d[:, 32:48, :].rearrange("t c C -> c t C"),
                )

            NCC = C // 16  # 3 x-chunks of 16 channels
            for p in range(np_blocks):
                for q in range(nq_blocks):
                    # ---- load window stack: 3 chunks [h | (c16, w64)] ----
                    xch = []
                    for cc in range(NCC):
                        xt = xin.tile([64, 16, 64], F32R, tag="x")
                        nc.sync.dma_start(
                            out=xt,
                            in_=x[
                                16 * cc : 16 * cc + 16,
                                64 * p : 64 * p + 64,
                                64 * q : 64 * q + 64,
                            ].rearrange("c h w -> h c w"),
                        )
                        xch.append(xt)

                    # ---- h-stage + swap1 ----
                    # V[t][part = 32*Hh + wsub | (c64, wb2, hsub32)]; c 48..63 dup junk
                    vt_tiles = []
                    for t in range(T):
                        vt = vbuf.tile([64, 64, 2, 32], F32, tag="v")
                        for yc in range(2 * NCC):
                            cc, sub = yc // 2, yc % 2
                            y = ypsum.tile([64, 8, 64], F32, tag="y")
                            nc.tensor.matmul(
                                y,
                                hs_sb[:, t, :],
                                xch[cc][:, 8 * sub : 8 * sub + 8, :],
                                start=True,
                                stop=True,
                            )
                            # swap1: blocks (c8, wb2); partitions get wsub
                            nc.vector.transpose(
                                out=vt[:, 8 * yc : 8 * yc + 8, :, :], in_=y
                            )
                            if yc >= 4:  # duplicate c 32..47 into pad slots 48..63
                                nc.vector.transpose(
                                    out=vt[:, 8 * yc + 16 : 8 * yc + 24, :, :], in_=y
                                )
                        v2 = vbuf.tile([64, 64, 2, 32], F32R, tag="v2")
                        nc.any.tensor_copy(out=v2, in_=vt)
                        vt_tiles.append(v2)

                    # ---- per H'-half: w-stage, swap2, c-stage ----
                    for hh in range(2):
                        ua = {}
                        ub = {}
                        for t in range(T):
                            # four 1-bank psum tiles: (stream, hq)
                            za = [
                                zpsum.tile([64, 16, 32], F32, tag="z", name=f"za{i}")
                                for i in range(2)
                            ]
                            zb = [
                                zpsum.tile([64, 16, 32], F32, tag="z", name=f"zb{i}")
                                for i in range(2)
                            ]
                            for wb in range(2):
                                lhs = ws_sb[32 * hh : 32 * hh + 32, t, wb, :]
                                rhs_a = vt_tiles[t][
                                    32 * hh : 32 * hh + 32, 0:32, wb, :
                                ].rearrange("p c h -> p h c")
                                rhs_b = vt_tiles[t][
                                    32 * hh : 32 * hh + 32, 32:64, wb, :
                                ].rearrange("p c h -> p h c")
                                for hq in range(2):
                                    nc.tensor.matmul(
                                        za[hq],
                                        lhs,
                                        rhs_a[:, 16 * hq : 16 * hq + 16, :],
                                        start=(wb == 0),
                                        stop=(wb == 1),
                                        tile_position=(32 * hh, 0),
                                    )
                                    nc.tensor.matmul(
                                        zb[hq],
                                        lhs,
                                        rhs_b[:, 16 * hq : 16 * hq + 16, :],
                                        start=(wb == 0),
                                        stop=(wb == 1),
                                        tile_position=(32 * hh, 0),
                                    )
                            # swap2: partitions get c-sub32
                            uat = ubuf.tile([64, 32, 32], F32, tag="ua")
                            ubt = ubuf.tile([64, 32, 32], F32, tag="ub")
                            for hq in range(2):
                                nc.vector.transpose(
                                    out=uat[:, 16 * hq : 16 * hq + 16, :], in_=za[hq]
                                )
                                nc.vector.transpose(
                                    out=ubt[:, 16 * hq : 16 * hq + 16, :], in_=zb[hq]
                                )
                            uat2 = ubuf.tile([64, 32, 32], F32R, tag="ua2")
                            nc.any.tensor_copy(out=uat2, in_=uat)
                            ubt2 = ubuf.tile([64, 32, 32], F32R, tag="ub2")
                            nc.any.tensor_copy(out=ubt2, in_=ubt)
                            ua[t] = uat2
                            ub[t] = ubt2

                        for hq in range(2):
                            # free = (vb, hsub, wsub): each vb-half is one psum bank
                            o_ps = opsum.tile([C, 2, 16, 32], F32, tag="o")
                            chain = []
                            for t in range(T):
                                chain.append((csa_sb, ua[t], t))
                                chain.append((csb_sb, ub[t], t))
                            for vb in range(2):
                                for ci, (cmat, u, t) in enumerate(chain):
                                    nc.tensor.matmul(
                                        o_ps[:, vb, :, :],
                                        cmat[32 * vb : 32 * vb + 32, t, :],
                                        u[
                                            32 * vb : 32 * vb + 32,
                                            16 * hq : 16 * hq + 16,
                                            :,
                                        ],
                                        start=(ci == 0),
                                        stop=(ci == len(chain) - 1),
                                        tile_position=(32 * vb, 0),
                                    )
                            o_sb = obuf.tile([C, 16, 2, 32], F32, tag="os")
                            # reorder (vb, hsub, wsub) -> (hsub, vb, wsub) during the exit copy
                            nc.any.tensor_copy(
                                out=o_sb.rearrange("p h v w -> p v h w"), in_=o_ps
                            )
                            r0 = 64 * p + 32 * hh + 16 * hq
                            nc.sync.dma_start(
                                out=out[:, r0 : r0 + 16, 64 * q : 64 * q + 64],
                                in_=o_sb,
                            )

    nc.compile()
    return nc


def _get_nc(np_blocks, nq_blocks):
    key = (np_blocks, nq_blocks)
    if key not in _CACHE:
        _CACHE[key] = _build(np_blocks, nq_blocks)
    return _CACHE[key]


def kernel(x, Ws, Hs, Cs, window_size):
    global LAST_EXEC_NS
    from concourse.bass_utils import run_bass_kernel_spmd

    x = np.asarray(x, dtype=np.float32)
    Ws = np.asarray(Ws, dtype=np.float32)
    Hs = np.asarray(Hs, dtype=np.float32)
    Cs = np.asarray(Cs, dtype=np.float32)
    assert int(window_size) == WS
    assert x.shape == (B, C, H, W)

    nc = _get_nc(4, 8)
    in_maps = []
    for core in range(NCORES):
        b, ph = core // 2, core % 2
        shard = np.ascontiguousarray(x[b, :, ph * PH : (ph + 1) * PH, :])
        in_maps.append({"x": shard, "hs": Hs, "ws": Ws, "cs": Cs})

    res = run_bass_kernel_spmd(nc, in_maps, core_ids=list(range(NCORES)))
    LAST_EXEC_NS = res.exec_time_ns

    full = np.empty((B, C, H, W), dtype=np.float32)
    for core in range(NCORES):
        b, ph = core // 2, core % 2
        full[b, :, ph * PH : (ph + 1) * PH, :] = res.results[core]["out"]
    return full



# revision 7
# speedup vs baseline: 1.7865x; 1.7865x over previous
"""FeatureProcessingBlock Trainium kernel (bf16 pipeline, v2).

out = sum_t einsum('bcphqw,twW,thH,tcC->bCpHqW', x.reshape(B,C,P,64,Q,64), Ws, Hs, Cs)

Sharding: 8 cores = (B=4) x (H-halves=2); each core gets x[b, :, ph*256:(ph+1)*256, :]
i.e. a [C=48, 256, 512] slab = 4 p-blocks x 4 double-windows (dw = two adjacent
64x64 windows sharing a 128-wide w-slab).

Per-core pipeline, all matmul operands bf16 (PSUM accumulates f32):
  c-stage  (data-stationary): lhsT = x[:, h, 128w-slab] [48c, 128w] (FWL),
            rhs = Cstack [48c, (t3, c'48)]  ->  PSUM [128w, (t, c')]
            -> Scalar drain/cast to Ybuf [128 (win,w), (t, j24, cs2, h64)]
  w-stage  (factor-stationary): lhsT = blkdiag(Ws_t, Ws_t) [128, 128],
            rhs = Ybuf[:, t, 3j-chunk]  ->  Z_t PSUM [128 (win, W'), (j, cs, h)]
            -> Vector drain to Zsb bf16
  T-stage  (PE-array transpose): [128, 128] blocks of Zsb -> ZT [128 (cs,h), (win,W')]
            -> GpSimd drain to ZTbuf bf16
  h-stage  (t-sum in PSUM): lhsT = blkdiag(Hs_t, Hs_t), rhs = ZTbuf[t] chunks,
            3-matmul accumulation  ->  O [128 (cs, H'), (j8, W')]
            -> Vector drain f32 -> DMA out (c-stride-2 rows)
"""

import numpy as np

B, C, H, W = 4, 48, 512, 512
T, WS = 3, 64
NCORES = 8
PH = H // 2    # 256 rows per core
NP = PH // 64  # 4 p-blocks
ND = W // 128  # 4 double-windows per p-block

LAST_EXEC_NS = None
_CACHE = {}


def _build():
    import concourse.bacc as bacc
    import concourse.mybir as mybir
    from concourse.bass import MemorySpace
    from concourse.tile import TileContext

    F32 = mybir.dt.float32
    BF16 = mybir.dt.bfloat16

    nc = bacc.Bacc("TRN2", target_bir_lowering=False, debug=False, num_devices=NCORES)
    x = nc.dram_tensor("x", [C, PH, W], BF16, kind="ExternalInput")
    cstk = nc.dram_tensor("cstk", [C, T * C], BF16, kind="ExternalInput")
    wblk = nc.dram_tensor("wblk", [T, 128, 128], BF16, kind="ExternalInput")
    hblk = nc.dram_tensor("hblk", [T, 128, 128], BF16, kind="ExternalInput")
    iden = nc.dram_tensor("iden", [128, 128], BF16, kind="ExternalInput")
    out = nc.dram_tensor("out", [C, PH, W], F32, kind="ExternalOutput")

    with TileContext(nc) as tc:
        with (
            tc.tile_pool(name="consts", bufs=1) as consts,
            tc.tile_pool(name="xin", bufs=3) as xin,
            tc.tile_pool(name="ybuf", bufs=2) as ypool,
            tc.tile_pool(name="zbuf", bufs=2) as zpool,
            tc.tile_pool(name="ztbuf", bufs=2) as ztpool,
            tc.tile_pool(name="obuf", bufs=2) as opool,
            tc.tile_pool(name="cps", bufs=2, space=MemorySpace.PSUM) as cps,
            tc.tile_pool(name="zps", bufs=2, space=MemorySpace.PSUM) as zps,
            tc.tile_pool(name="tps", bufs=2, space=MemorySpace.PSUM) as tps,
            tc.tile_pool(name="ops", bufs=2, space=MemorySpace.PSUM) as ops,
        ):
            cstk_sb = consts.tile([C, T, C], BF16)
            nc.sync.dma_start(
                out=cstk_sb, in_=cstk[:, :].rearrange("c (t d) -> c t d", t=T)
            )
            wblk_sb = consts.tile([128, T, 128], BF16)
            nc.sync.dma_start(out=wblk_sb, in_=wblk[:, :, :].rearrange("t k m -> k t m"))
            hblk_sb = consts.tile([128, T, 128], BF16)
            nc.sync.dma_start(out=hblk_sb, in_=hblk[:, :, :].rearrange("t k m -> k t m"))
            iden_sb = consts.tile([128, 128], BF16)
            nc.sync.dma_start(out=iden_sb, in_=iden[:, :])

            for p in range(NP):
                for d in range(ND):
                    # ---- load dw: [48c, 64h, 128w] ----
                    xt = xin.tile([C, 64, 128], BF16, tag="x")
                    nc.sync.dma_start(
                        out=xt, in_=x[:, 64 * p : 64 * p + 64, 128 * d : 128 * d + 128]
                    )

                    # ---- c-stage ----
                    yb = ypool.tile([128, T, 24, 2, 64], BF16, tag="y")
                    for hh in range(32):
                        cp = cps.tile([128, 2, T, 24, 2], F32, tag="c")
                        for i in range(2):
                            nc.tensor.matmul(
                                cp[:, i],
                                lhsT=xt[:, 2 * hh + i, :],
                                rhs=cstk_sb,
                                start=True,
                                stop=True,
                            )
                        nc.scalar.copy(
                            out=yb[:, :, :, :, 2 * hh : 2 * hh + 2].rearrange(
                                "p t j s h -> p h t j s"
                            ),
                            in_=cp,
                        )

                    # ---- w-stage + transpose per t ----
                    ztbs = []
                    for t in range(T):
                        zb = zpool.tile([128, 24, 2, 64], BF16, tag=f"z{t}")
                        for jj in range(8):
                            zp = zps.tile([128, 3, 2, 64], F32, tag="z")
                            nc.tensor.matmul(
                                zp,
                                lhsT=wblk_sb[:, t, :],
                                rhs=yb[:, t, 3 * jj : 3 * jj + 3, :, :],
                                start=True,
                                stop=True,
                            )
                            nc.vector.tensor_copy(
                                out=zb[:, 3 * jj : 3 * jj + 3, :, :], in_=zp
                            )
                        ztb = ztpool.tile([128, 24, 128], BF16, tag=f"zt{t}")
                        for j in range(24):
                            tp = tps.tile([128, 128], BF16, tag="t")
                            nc.tensor.transpose(tp, zb[:, j, :, :], iden_sb)
                            nc.vector.tensor_copy(out=ztb[:, j, :], in_=tp)
                        ztbs.append(ztb)

                    # ---- h-stage (t-sum in PSUM) + out ----
                    ob = opool.tile([128, 2, 3, 8, 64], F32, tag="ob")
                    for win in range(2):
                        for cc in range(3):
                            op = ops.tile([128, 8, 64], F32, tag="o")
                            for t in range(T):
                                nc.tensor.matmul(
                                    op,
                                    lhsT=hblk_sb[:, t, :],
                                    rhs=ztbs[t][
                                        :, 8 * cc : 8 * cc + 8, 64 * win : 64 * win + 64
                                    ],
                                    start=(t == 0),
                                    stop=(t == T - 1),
                                )
                            nc.scalar.copy(out=ob[:, win, cc], in_=op)
                    for win in range(2):
                        for cs in range(2):
                            col0 = 128 * d + 64 * win
                            nc.sync.dma_start(
                                out=out[
                                    cs : C : 2, 64 * p : 64 * p + 64, col0 : col0 + 64
                                ].rearrange("c h w -> h c w"),
                                in_=ob[64 * cs : 64 * cs + 64, win, :, :, :].rearrange(
                                    "p a j w -> p (a j) w"
                                ),
                            )

    nc.compile()
    return nc


def _get_nc():
    if "nc" not in _CACHE:
        _CACHE["nc"] = _build()
    return _CACHE["nc"]


def _prep_consts(Ws, Hs, Cs):
    import ml_dtypes

    bf = ml_dtypes.bfloat16
    cstk = np.ascontiguousarray(Cs.transpose(1, 0, 2).reshape(C, T * C)).astype(bf)
    wblk = np.zeros((T, 128, 128), np.float32)
    hblk = np.zeros((T, 128, 128), np.float32)
    for t in range(T):
        wblk[t, 0:64, 0:64] = Ws[t]
        wblk[t, 64:128, 64:128] = Ws[t]
        hblk[t, 0:64, 0:64] = Hs[t]
        hblk[t, 64:128, 64:128] = Hs[t]
    iden = np.eye(128, dtype=np.float32)
    return cstk, wblk.astype(bf), hblk.astype(bf), iden.astype(bf)


def kernel(x, Ws, Hs, Cs, window_size):
    global LAST_EXEC_NS
    import ml_dtypes
    from concourse.bass_utils import run_bass_kernel_spmd

    bf = ml_dtypes.bfloat16
    x = np.asarray(x, dtype=np.float32)
    Ws = np.asarray(Ws, dtype=np.float32)
    Hs = np.asarray(Hs, dtype=np.float32)
    Cs = np.asarray(Cs, dtype=np.float32)
    assert int(window_size) == WS
    assert x.shape == (B, C, H, W)

    nc = _get_nc()
    cstk, wblk, hblk, iden = _prep_consts(Ws, Hs, Cs)
    xb = x.astype(bf)

    in_maps = []
    for core in range(NCORES):
        b, ph = core // 2, core % 2
        shard = np.ascontiguousarray(xb[b, :, ph * PH : (ph + 1) * PH, :])
        in_maps.append(
            {"x": shard, "cstk": cstk, "wblk": wblk, "hblk": hblk, "iden": iden}
        )

    res = run_bass_kernel_spmd(nc, in_maps, core_ids=list(range(NCORES)))
    LAST_EXEC_NS = res.exec_time_ns

    full = np.empty((B, C, H, W), dtype=np.float32)
    for core in range(NCORES):
        b, ph = core // 2, core % 2
        full[b, :, ph * PH : (ph + 1) * PH, :] = res.results[core]["out"]
    return full


# revision 13
# speedup vs baseline: 3.2512x; 1.8199x over previous
"""FeatureProcessingBlock Trainium kernel (bf16 pipeline, v3).

out = sum_t einsum('bcphqw,twW,thH,tcC->bCpHqW', x.reshape(B,C,P,64,Q,64), Ws, Hs, Cs)

Sharding: 8 cores = (B=4) x (H-halves=2); each core gets x[b, :, ph*256:(ph+1)*256, :]
i.e. a [C=48, 256, 512] slab = 4 p-blocks x 4 double-windows (dw = two adjacent
64x64 windows sharing a 128-wide w-slab).

Per-core pipeline, all matmul operands bf16 (PSUM accumulates f32):
  c-stage  (data-stationary, h-pair packed): lhsT = x[(par,c)96, wp128] chunk,
            rhs = cstk_lo / cstk_hi [96, 144] (zero-padded halves select parity)
            -> PSUM [128 wp, (2 h, t3, c'48)]
            -> Scalar drain/cast (contiguous) to Ybuf [128 (win,w), (h64, t3, j24, cs2)]
  wT-stage (fused w-matmul + transpose; data-stationary): lhsT = Ybuf (t,j)-chunk
            [128 (win,w), 128 (h,cs)], rhs = blkdiag(Ws_t, Ws_t)
            -> PSUM [128 (h,cs), (win, W')] -- already transposed for the h-stage
            -> Vector drain/cast to ZT[t] bf16
  h-stage  (t-sum in PSUM): lhsT = Hblk2[t] (rows 2h+cs, cols (cs,H')),
            rhs = ZT[t] chunks, 3-matmul accumulation
            -> O [128 (cs, H'), (j8, W')] -> Vector drain f32 -> DMA out
"""

import numpy as np

B, C, H, W = 4, 48, 512, 512
T, WS = 3, 64
NCORES = 8
PH = H // 2    # 256 rows per core
NP = PH // 64  # 4 p-blocks
NDD = W // 256  # 2 dw-pairs per p-block

LAST_EXEC_NS = None
_CACHE = {}


def _build():
    import concourse.bacc as bacc
    import concourse.mybir as mybir
    from concourse.bass import MemorySpace
    from concourse.tile import TileContext

    F32 = mybir.dt.float32
    BF16 = mybir.dt.bfloat16

    nc = bacc.Bacc("TRN2", target_bir_lowering=False, debug=False, num_devices=NCORES)
    x = nc.dram_tensor("x", [C, PH, W], BF16, kind="ExternalInput")
    cstk = nc.dram_tensor("cstk", [2, 96, T * C], BF16, kind="ExternalInput")
    wblk = nc.dram_tensor("wblk", [T, 128, 128], BF16, kind="ExternalInput")
    hblk = nc.dram_tensor("hblk", [T, 128, 128], BF16, kind="ExternalInput")
    out = nc.dram_tensor("out", [C, PH, W], F32, kind="ExternalOutput")

    with TileContext(nc) as tc:
        with (
            tc.tile_pool(name="consts", bufs=1) as consts,
            tc.tile_pool(name="xin", bufs=2) as xin,
            tc.tile_pool(name="ybuf", bufs=2) as ypool,
            tc.tile_pool(name="ztbuf", bufs=2) as ztpool,
            tc.tile_pool(name="obuf", bufs=2) as opool,
            tc.tile_pool(name="cps", bufs=2, space=MemorySpace.PSUM) as cps,
            tc.tile_pool(name="tps", bufs=3, space=MemorySpace.PSUM) as tps,
            tc.tile_pool(name="ops", bufs=2, space=MemorySpace.PSUM) as ops,
        ):
            cstk_sb = consts.tile([96, 2, T, C], BF16)
            nc.sync.dma_start(
                out=cstk_sb,
                in_=cstk[:, :, :].rearrange("par k (t d) -> k par t d", t=T),
            )
            wblk_sb = consts.tile([128, T, 128], BF16)
            nc.sync.dma_start(out=wblk_sb, in_=wblk[:, :, :].rearrange("t k m -> k t m"))
            hblk_sb = consts.tile([128, T, 128], BF16)
            nc.sync.dma_start(out=hblk_sb, in_=hblk[:, :, :].rearrange("t k m -> k t m"))

            for p in range(NP):
                for dd in range(NDD):
                    # ---- load dw-pair: [(par2, c48), 32 hh, 256 wp] ----
                    xt = xin.tile([96, 32, 256], BF16, tag="x")
                    for par in range(2):
                        nc.sync.dma_start(
                            out=xt[48 * par : 48 * par + 48],
                            in_=x[
                                :,
                                64 * p + par : 64 * p + 64 : 2,
                                256 * dd : 256 * dd + 256,
                            ],
                        )
                    for dhalf in range(2):
                        wp0 = 128 * dhalf
                        d = 2 * dd + dhalf

                        # ---- c-stage ----
                        # Ybuf [128 (win,w), (t3, j24, h64, cs2)]
                        yb = ypool.tile([128, T, 24, 64, 2], BF16, tag="y")
                        for hh in range(32):
                            cp = cps.tile([128, 2, T, 24, 2], F32, tag="c")
                            for par in range(2):
                                nc.tensor.matmul(
                                    cp[:, par],
                                    lhsT=xt[:, hh, wp0 : wp0 + 128],
                                    rhs=cstk_sb[:, par],
                                    start=True,
                                    stop=True,
                                )
                            nc.scalar.copy(
                                out=yb[:, :, :, 2 * hh : 2 * hh + 2, :],
                                in_=cp.rearrange("p par t j s -> p t j par s"),
                            )

                        # ---- fused w+transpose stage per t ----
                        ztbs = []
                        for t in range(T):
                            ztb = ztpool.tile([128, 24, 128], BF16, tag=f"zt{t}")
                            for jp in range(12):
                                tp = tps.tile([128, 2, 128], F32, tag="t")
                                for i in range(2):
                                    nc.tensor.matmul(
                                        tp[:, i],
                                        lhsT=yb[:, t, 2 * jp + i],
                                        rhs=wblk_sb[:, t, :],
                                        start=True,
                                        stop=True,
                                    )
                                nc.vector.tensor_copy(
                                    out=ztb[:, 2 * jp : 2 * jp + 2, :], in_=tp
                                )
                            ztbs.append(ztb)

                        # ---- h-stage (t-sum in PSUM) + out ----
                        ob = opool.tile([128, 2, 3, 8, 64], F32, tag="ob")
                        for win in range(2):
                            for cc in range(3):
                                op = ops.tile([128, 8, 64], F32, tag="o")
                                for t in range(T):
                                    nc.tensor.matmul(
                                        op,
                                        lhsT=hblk_sb[:, t, :],
                                        rhs=ztbs[t][
                                            :,
                                            8 * cc : 8 * cc + 8,
                                            64 * win : 64 * win + 64,
                                        ],
                                        start=(t == 0),
                                        stop=(t == T - 1),
                                    )
                                nc.vector.tensor_copy(out=ob[:, win, cc], in_=op)
                        for win in range(2):
                            for cs in range(2):
                                col0 = 128 * d + 64 * win
                                nc.sync.dma_start(
                                    out=out[
                                        cs : C : 2,
                                        64 * p : 64 * p + 64,
                                        col0 : col0 + 64,
                                    ].rearrange("c h w -> h c w"),
                                    in_=ob[64 * cs : 64 * cs + 64, win].rearrange(
                                        "p a j w -> p (a j) w"
                                    ),
                                )

    nc.compile()
    return nc


def _get_nc():
    if "nc" not in _CACHE:
        _CACHE["nc"] = _build()
    return _CACHE["nc"]


def _prep_consts(Ws, Hs, Cs):
    import ml_dtypes

    bf = ml_dtypes.bfloat16
    # cstk[par] = [96, 144]: rows par*48..par*48+48 hold Cstack, rest zero
    cstack = Cs.transpose(1, 0, 2).reshape(C, T * C)  # [48, 144]
    cstk = np.zeros((2, 96, T * C), np.float32)
    cstk[0, 0:48] = cstack
    cstk[1, 48:96] = cstack
    wblk = np.zeros((T, 128, 128), np.float32)
    hblk = np.zeros((T, 128, 128), np.float32)
    for t in range(T):
        wblk[t, 0:64, 0:64] = Ws[t]
        wblk[t, 64:128, 64:128] = Ws[t]
        # rows p = 2h+cs, cols m = cs*64+g
        for cs in range(2):
            hblk[t, cs::2, cs * 64 : cs * 64 + 64] = Hs[t]
    return cstk.astype(bf), wblk.astype(bf), hblk.astype(bf)


def kernel(x, Ws, Hs, Cs, window_size):
    global LAST_EXEC_NS
    import ml_dtypes
    from concourse.bass_utils import run_bass_kernel_spmd

    bf = ml_dtypes.bfloat16
    x = np.asarray(x, dtype=np.float32)
    Ws = np.asarray(Ws, dtype=np.float32)
    Hs = np.asarray(Hs, dtype=np.float32)
    Cs = np.asarray(Cs, dtype=np.float32)
    assert int(window_size) == WS
    assert x.shape == (B, C, H, W)

    nc = _get_nc()
    cstk, wblk, hblk = _prep_consts(Ws, Hs, Cs)
    xb = x.astype(bf)

    in_maps = []
    for core in range(NCORES):
        b, ph = core // 2, core % 2
        shard = np.ascontiguousarray(xb[b, :, ph * PH : (ph + 1) * PH, :])
        in_maps.append({"x": shard, "cstk": cstk, "wblk": wblk, "hblk": hblk})

    res = run_bass_kernel_spmd(nc, in_maps, core_ids=list(range(NCORES)))
    LAST_EXEC_NS = res.exec_time_ns

    full = np.empty((B, C, H, W), dtype=np.float32)
    for core in range(NCORES):
        b, ph = core // 2, core % 2
        full[b, :, ph * PH : (ph + 1) * PH, :] = res.results[core]["out"]
    return full


# revision 14
# speedup vs baseline: 4.6720x; 1.4370x over previous
"""FeatureProcessingBlock Trainium kernel (bf16 pipeline, v4).

out = sum_t einsum('bcphqw,twW,thH,tcC->bCpHqW', x.reshape(B,C,P,64,Q,64), Ws, Hs, Cs)

Sharding: 8 cores = (B=4) x (H-halves=2); each core gets x[b, :, ph*256:(ph+1)*256, :]
a [C=48, 256, 512] slab = 4 p-blocks x 4 double-windows (dw = two adjacent
64x64 windows in a 128-wide w-slab).

Per-core pipeline, all matmul operands bf16 (PSUM accumulates f32):
  c-stage  (data-stationary, h-pair packed): lhsT = x[(par,c)96, wp128] chunk,
            rhs = [cstk_lo | cstk_hi] [96, 288] (zero-padded halves per parity)
            -> PSUM [128 wp, (par2, t3, j24, cs2)]
            -> Scalar drain/cast to Ybuf [128 (win,w), (t3, j24, h64, cs2)]
  wT-stage (fused w-matmul + transpose; data-stationary): lhsT = Ybuf (t,j)-chunk
            [128 (win,w), 128 (h,cs)], rhs = blkdiag(Ws_t, Ws_t)
            -> PSUM [128 (h,cs), (win, W')] -- already transposed for the h-stage
            -> Vector/Scalar drain/cast to ZT[t] bf16
  h-stage  (t-sum in PSUM): lhsT = Hblk2[t] (rows 2h+cs, cols (cs,H')),
            rhs = ZT[t] chunks, 3-matmul accumulation
            -> O [128 (cs, H'), (j8, W')] -> Vector drain f32 -> DMA out
"""

import numpy as np

B, C, H, W = 4, 48, 512, 512
T, WS = 3, 64
NCORES = 8
PH = H // 2    # 256 rows per core
NP = PH // 64  # 4 p-blocks

LAST_EXEC_NS = None
_CACHE = {}


def _build():
    import concourse.bacc as bacc
    import concourse.mybir as mybir
    from concourse.bass import MemorySpace
    from concourse.tile import TileContext

    F32 = mybir.dt.float32
    BF16 = mybir.dt.bfloat16

    nc = bacc.Bacc("TRN2", target_bir_lowering=False, debug=False, num_devices=NCORES)
    x = nc.dram_tensor("x", [C, PH, W], BF16, kind="ExternalInput")
    cstk = nc.dram_tensor("cstk", [96, 2 * T * C], BF16, kind="ExternalInput")
    wblk = nc.dram_tensor("wblk", [T, 128, 128], BF16, kind="ExternalInput")
    hblk = nc.dram_tensor("hblk", [T, 128, 128], BF16, kind="ExternalInput")
    out = nc.dram_tensor("out", [C, PH, W], F32, kind="ExternalOutput")

    with TileContext(nc) as tc:
        with (
            tc.tile_pool(name="consts", bufs=1) as consts,
            tc.tile_pool(name="xin", bufs=2) as xin,
            tc.tile_pool(name="ybuf", bufs=2) as ypool,
            tc.tile_pool(name="ztbuf", bufs=2) as ztpool,
            tc.tile_pool(name="obuf", bufs=2) as opool,
            tc.tile_pool(name="cps", bufs=3, space=MemorySpace.PSUM) as cps,
            tc.tile_pool(name="tps", bufs=2, space=MemorySpace.PSUM) as tps,
            tc.tile_pool(name="ops", bufs=3, space=MemorySpace.PSUM) as ops,
        ):
            cstk_sb = consts.tile([96, 2, T, 24, 2], BF16)
            nc.sync.dma_start(
                out=cstk_sb,
                in_=cstk[:, :].rearrange(
                    "k (par t j s) -> k par t j s", par=2, t=T, j=24
                ),
            )
            wblk_sb = consts.tile([128, T, 128], BF16)
            nc.sync.dma_start(out=wblk_sb, in_=wblk[:, :, :].rearrange("t k m -> k t m"))
            hblk_sb = consts.tile([128, T, 128], BF16)
            nc.sync.dma_start(out=hblk_sb, in_=hblk[:, :, :].rearrange("t k m -> k t m"))

            for p in range(NP):
                # ---- load p-row: [(par2, c48), 32 hh, 512 w] (1KB runs) ----
                xt = xin.tile([96, 32, W], BF16, tag="x")
                for par in range(2):
                    nc.sync.dma_start(
                        out=xt[48 * par : 48 * par + 48],
                        in_=x[:, 64 * p + par : 64 * p + 64 : 2, :],
                    )
                for d in range(4):
                    wp0 = 128 * d

                    # ---- c-stage ----
                    # Ybuf [128 (win,w), (t3, j24, h64, cs2)]
                    yb = ypool.tile([128, T, 24, 64, 2], BF16, tag="y")
                    for hh in range(32):
                        cp = cps.tile([128, 2, T, 24, 2], F32, tag="c")
                        nc.tensor.matmul(
                            cp,
                            lhsT=xt[:, hh, wp0 : wp0 + 128],
                            rhs=cstk_sb,
                            start=True,
                            stop=True,
                        )
                        nc.scalar.copy(
                            out=yb[:, :, :, 2 * hh : 2 * hh + 2, :],
                            in_=cp.rearrange("p par t j s -> p t j par s"),
                        )

                    # ---- fused w+transpose stage per t ----
                    ztbs = []
                    for t in range(T):
                        ztb = ztpool.tile([128, 24, 128], BF16, tag=f"zt{t}")
                        for jq in range(6):
                            tp = tps.tile([128, 4, 128], F32, tag="t")
                            for i in range(4):
                                nc.tensor.matmul(
                                    tp[:, i],
                                    lhsT=yb[:, t, 4 * jq + i],
                                    rhs=wblk_sb[:, t, :],
                                    start=True,
                                    stop=True,
                                )
                            eng = nc.scalar if jq % 3 == 2 else nc.vector
                            if eng is nc.scalar:
                                nc.scalar.copy(
                                    out=ztb[:, 4 * jq : 4 * jq + 4, :], in_=tp
                                )
                            else:
                                nc.vector.tensor_copy(
                                    out=ztb[:, 4 * jq : 4 * jq + 4, :], in_=tp
                                )
                        ztbs.append(ztb)

                    # ---- h-stage (t-sum in PSUM) + out ----
                    ob = opool.tile([128, 3, 8, 2, 64], F32, tag="ob")
                    for win in range(2):
                        for cc in range(3):
                            op = ops.tile([128, 8, 64], F32, tag="o")
                            for t in range(T):
                                nc.tensor.matmul(
                                    op,
                                    lhsT=hblk_sb[:, t, :],
                                    rhs=ztbs[t][
                                        :,
                                        8 * cc : 8 * cc + 8,
                                        64 * win : 64 * win + 64,
                                    ],
                                    start=(t == 0),
                                    stop=(t == T - 1),
                                )
                            nc.vector.tensor_copy(out=ob[:, cc, :, win, :], in_=op)
                    for cs in range(2):
                        nc.sync.dma_start(
                            out=out[
                                cs : C : 2,
                                64 * p : 64 * p + 64,
                                128 * d : 128 * d + 128,
                            ].rearrange("c h w -> h c w"),
                            in_=ob[64 * cs : 64 * cs + 64].rearrange(
                                "p a j win w -> p (a j) (win w)"
                            ),
                        )

    nc.compile()
    return nc


def _get_nc():
    if "nc" not in _CACHE:
        _CACHE["nc"] = _build()
    return _CACHE["nc"]


def _prep_consts(Ws, Hs, Cs):
    import ml_dtypes

    bf = ml_dtypes.bfloat16
    # cstk [96, (par2, t3, c'48)]: rows par*48..+48 hold Cstack for that parity
    cstack = Cs.transpose(1, 0, 2).reshape(C, T * C)  # [48, (t, c')]
    cstk = np.zeros((96, 2, T * C), np.float32)
    cstk[0:48, 0] = cstack
    cstk[48:96, 1] = cstack
    cstk = cstk.reshape(96, 2 * T * C)
    wblk = np.zeros((T, 128, 128), np.float32)
    hblk = np.zeros((T, 128, 128), np.float32)
    for t in range(T):
        wblk[t, 0:64, 0:64] = Ws[t]
        wblk[t, 64:128, 64:128] = Ws[t]
        # rows p = 2h+cs, cols m = cs*64+g
        for cs in range(2):
            hblk[t, cs::2, cs * 64 : cs * 64 + 64] = Hs[t]
    return cstk.astype(bf), wblk.astype(bf), hblk.astype(bf)


def kernel(x, Ws, Hs, Cs, window_size):
    global LAST_EXEC_NS
    import ml_dtypes
    from concourse.bass_utils import run_bass_kernel_spmd

    bf = ml_dtypes.bfloat16
    x = np.asarray(x, dtype=np.float32)
    Ws = np.asarray(Ws, dtype=np.float32)
    Hs = np.asarray(Hs, dtype=np.float32)
    Cs = np.asarray(Cs, dtype=np.float32)
    assert int(window_size) == WS
    assert x.shape == (B, C, H, W)

    nc = _get_nc()
    cstk, wblk, hblk = _prep_consts(Ws, Hs, Cs)
    xb = x.astype(bf)

    in_maps = []
    for core in range(NCORES):
        b, ph = core // 2, core % 2
        shard = np.ascontiguousarray(xb[b, :, ph * PH : (ph + 1) * PH, :])
        in_maps.append({"x": shard, "cstk": cstk, "wblk": wblk, "hblk": hblk})

    res = run_bass_kernel_spmd(nc, in_maps, core_ids=list(range(NCORES)))
    LAST_EXEC_NS = res.exec_time_ns

    full = np.empty((B, C, H, W), dtype=np.float32)
    for core in range(NCORES):
        b, ph = core // 2, core % 2
        full[b, :, ph * PH : (ph + 1) * PH, :] = res.results[core]["out"]
    return full


# revision 21
# speedup vs baseline: 5.1829x; 1.1093x over previous
"""FeatureProcessingBlock Trainium kernel (bf16 pipeline, v4).

out = sum_t einsum('bcphqw,twW,thH,tcC->bCpHqW', x.reshape(B,C,P,64,Q,64), Ws, Hs, Cs)

Sharding: 8 cores = (B=4) x (H-halves=2); each core gets x[b, :, ph*256:(ph+1)*256, :]
a [C=48, 256, 512] slab = 4 p-blocks x 4 double-windows (dw = two adjacent
64x64 windows in a 128-wide w-slab).

Per-core pipeline, all matmul operands bf16 (PSUM accumulates f32):
  c-stage  (data-stationary, h-pair packed): lhsT = x[(par,c)96, wp128] chunk,
            rhs = [cstk_lo | cstk_hi] [96, 288] (zero-padded halves per parity)
            -> PSUM [128 wp, (par2, t3, j24, cs2)]
            -> Scalar drain/cast to Ybuf [128 (win,w), (t3, j24, h64, cs2)]
  wT-stage (fused w-matmul + transpose; data-stationary): lhsT = Ybuf (t,j)-chunk
            [128 (win,w), 128 (h,cs)], rhs = blkdiag(Ws_t, Ws_t)
            -> PSUM [128 (h,cs), (win, W')] -- already transposed for the h-stage
            -> Vector/Scalar drain/cast to ZT[t] bf16
  h-stage  (t-sum in PSUM): lhsT = Hblk2[t] (rows 2h+cs, cols (cs,H')),
            rhs = ZT[t] chunks, 3-matmul accumulation
            -> O [128 (cs, H'), (j8, W')] -> Vector drain f32 -> DMA out
"""

import numpy as np

B, C, H, W = 4, 48, 512, 512
T, WS = 3, 64
NCORES = 8
PH = H // 2    # 256 rows per core
NP = PH // 64  # 4 p-blocks

LAST_EXEC_NS = None
_CACHE = {}


def _build():
    import concourse.bacc as bacc
    import concourse.mybir as mybir
    from concourse.bass import MemorySpace
    from concourse.tile import TileContext

    F32 = mybir.dt.float32
    BF16 = mybir.dt.bfloat16

    nc = bacc.Bacc("TRN2", target_bir_lowering=False, debug=False, num_devices=NCORES)
    x = nc.dram_tensor("x", [C, PH, W], BF16, kind="ExternalInput")
    cstk = nc.dram_tensor("cstk", [96, 2 * T * C], BF16, kind="ExternalInput")
    wblk = nc.dram_tensor("wblk", [T, 128, 128], BF16, kind="ExternalInput")
    hblk = nc.dram_tensor("hblk", [T, 128, 128], BF16, kind="ExternalInput")
    out = nc.dram_tensor("out", [C, PH, W], BF16, kind="ExternalOutput")

    with TileContext(nc) as tc:
        with (
            tc.tile_pool(name="consts", bufs=1) as consts,
            tc.tile_pool(name="xin", bufs=2) as xin,
            tc.tile_pool(name="ybuf", bufs=2) as ypool,
            tc.tile_pool(name="ztbuf", bufs=2) as ztpool,
            tc.tile_pool(name="obuf", bufs=2) as opool,
            tc.tile_pool(name="cps", bufs=3, space=MemorySpace.PSUM) as cps,
            tc.tile_pool(name="tps", bufs=2, space=MemorySpace.PSUM) as tps,
            tc.tile_pool(name="ops", bufs=3, space=MemorySpace.PSUM) as ops,
        ):
            cstk_sb = consts.tile([96, T, 24, 2, 2], BF16)
            nc.sync.dma_start(
                out=cstk_sb,
                in_=cstk[:, :].rearrange(
                    "k (t j par s) -> k t j par s", t=T, j=24, par=2
                ),
            )
            wblk_sb = consts.tile([128, T, 128], BF16)
            nc.sync.dma_start(out=wblk_sb, in_=wblk[:, :, :].rearrange("t k m -> k t m"))
            hblk_sb = consts.tile([128, T, 128], BF16)
            nc.sync.dma_start(out=hblk_sb, in_=hblk[:, :, :].rearrange("t k m -> k t m"))

            for p in range(NP):
                # ---- load p-row: [(par2, c48), 32 hh, 512 w] (1KB runs) ----
                xt = xin.tile([96, 32, W], BF16, tag="x")
                for par in range(2):
                    nc.sync.dma_start(
                        out=xt[48 * par : 48 * par + 48],
                        in_=x[:, 64 * p + par : 64 * p + 64 : 2, :],
                    )
                ob = opool.tile([128, 3, 8, 4, 2, 64], BF16, tag="ob")
                for d in range(4):
                    wp0 = 128 * d

                    # ---- c-stage ----
                    # Ybuf [128 (win,w), (t3, j24, h64, cs2)]
                    yb = ypool.tile([128, T, 24, 64, 2], BF16, tag="y")
                    for hh in range(32):
                        cp = cps.tile([128, T, 24, 2, 2], F32, tag="c")
                        nc.tensor.matmul(
                            cp,
                            lhsT=xt[:, hh, wp0 : wp0 + 128],
                            rhs=cstk_sb,
                            start=True,
                            stop=True,
                        )
                        dst = yb[:, :, :, 2 * hh : 2 * hh + 2, :]
                        if hh % 4 == 3:
                            nc.vector.tensor_copy(out=dst, in_=cp)
                        else:
                            nc.scalar.copy(out=dst, in_=cp)

                    # ---- fused w+transpose stage per t ----
                    ztbs = []
                    for t in range(T):
                        ztb = ztpool.tile([128, 24, 128], BF16, tag=f"zt{t}")
                        for jq in range(6):
                            tp = tps.tile([128, 4, 128], F32, tag="t")
                            for i in range(4):
                                nc.tensor.matmul(
                                    tp[:, i],
                                    lhsT=yb[:, t, 4 * jq + i],
                                    rhs=wblk_sb[:, t, :],
                                    start=True,
                                    stop=True,
                                )
                            eng = nc.scalar if jq % 3 == 2 else nc.vector
                            if eng is nc.scalar:
                                nc.scalar.copy(
                                    out=ztb[:, 4 * jq : 4 * jq + 4, :], in_=tp
                                )
                            else:
                                nc.vector.tensor_copy(
                                    out=ztb[:, 4 * jq : 4 * jq + 4, :], in_=tp
                                )
                        ztbs.append(ztb)

                    # ---- h-stage (t-sum in PSUM) ----
                    for win in range(2):
                        for cc in range(3):
                            op = ops.tile([128, 8, 64], F32, tag="o")
                            for t in range(T):
                                nc.tensor.matmul(
                                    op,
                                    lhsT=hblk_sb[:, t, :],
                                    rhs=ztbs[t][
                                        :,
                                        8 * cc : 8 * cc + 8,
                                        64 * win : 64 * win + 64,
                                    ],
                                    start=(t == 0),
                                    stop=(t == T - 1),
                                )
                            nc.vector.tensor_copy(
                                out=ob[:, cc, :, d, win, :], in_=op
                            )
                # ---- out DMA per p-row (1KB runs) ----
                for cs in range(2):
                    nc.sync.dma_start(
                        out=out[cs : C : 2, 64 * p : 64 * p + 64, :].rearrange(
                            "c h w -> h c w"
                        ),
                        in_=ob[64 * cs : 64 * cs + 64].rearrange(
                            "p a j d win w -> p (a j) (d win w)"
                        ),
                    )

    nc.compile()
    return nc


def _get_nc():
    if "nc" not in _CACHE:
        _CACHE["nc"] = _build()
    return _CACHE["nc"]


def _prep_consts(Ws, Hs, Cs):
    import ml_dtypes

    bf = ml_dtypes.bfloat16
    # cstk [96, (t3, j24, par2, cs2)]: rows par*48..+48 hold Cs for that parity
    cstk = np.zeros((96, T, 24, 2, 2), np.float32)
    for par in range(2):
        # cols (t, j, par, cs) = Cs[t, c, c'=2j+cs]
        cstk[48 * par : 48 * par + 48, :, :, par, :] = Cs.transpose(1, 0, 2).reshape(
            C, T, 24, 2
        )
    cstk = cstk.reshape(96, 2 * T * C)
    wblk = np.zeros((T, 128, 128), np.float32)
    hblk = np.zeros((T, 128, 128), np.float32)
    for t in range(T):
        wblk[t, 0:64, 0:64] = Ws[t]
        wblk[t, 64:128, 64:128] = Ws[t]
        # rows p = 2h+cs, cols m = cs*64+g
        for cs in range(2):
            hblk[t, cs::2, cs * 64 : cs * 64 + 64] = Hs[t]
    return cstk.astype(bf), wblk.astype(bf), hblk.astype(bf)


def kernel(x, Ws, Hs, Cs, window_size):
    global LAST_EXEC_NS
    import ml_dtypes
    from concourse.bass_utils import run_bass_kernel_spmd

    bf = ml_dtypes.bfloat16
    x = np.asarray(x, dtype=np.float32)
    Ws = np.asarray(Ws, dtype=np.float32)
    Hs = np.asarray(Hs, dtype=np.float32)
    Cs = np.asarray(Cs, dtype=np.float32)
    assert int(window_size) == WS
    assert x.shape == (B, C, H, W)

    nc = _get_nc()
    cstk, wblk, hblk = _prep_consts(Ws, Hs, Cs)
    xb = x.astype(bf)

    in_maps = []
    for core in range(NCORES):
        b, ph = core // 2, core % 2
        shard = np.ascontiguousarray(xb[b, :, ph * PH : (ph + 1) * PH, :])
        in_maps.append({"x": shard, "cstk": cstk, "wblk": wblk, "hblk": hblk})

    res = run_bass_kernel_spmd(nc, in_maps, core_ids=list(range(NCORES)))
    LAST_EXEC_NS = res.exec_time_ns

    full = np.empty((B, C, H, W), dtype=np.float32)
    for core in range(NCORES):
        b, ph = core // 2, core % 2
        full[b, :, ph * PH : (ph + 1) * PH, :] = res.results[core]["out"].astype(
            np.float32
        )
    return full


# revision 24
# speedup vs baseline: 5.4064x; 1.0431x over previous
"""FeatureProcessingBlock Trainium kernel (bf16 pipeline, v4).

out = sum_t einsum('bcphqw,twW,thH,tcC->bCpHqW', x.reshape(B,C,P,64,Q,64), Ws, Hs, Cs)

Sharding: 8 cores = (B=4) x (H-halves=2); each core gets x[b, :, ph*256:(ph+1)*256, :]
a [C=48, 256, 512] slab = 4 p-blocks x 4 double-windows (dw = two adjacent
64x64 windows in a 128-wide w-slab).

Per-core pipeline, all matmul operands bf16 (PSUM accumulates f32):
  c-stage  (data-stationary, h-pair packed): lhsT = x[(par,c)96, wp128] chunk,
            rhs = [cstk_lo | cstk_hi] [96, 288] (zero-padded halves per parity)
            -> PSUM [128 wp, (par2, t3, j24, cs2)]
            -> Scalar drain/cast to Ybuf [128 (win,w), (t3, j24, h64, cs2)]
  wT-stage (fused w-matmul + transpose; data-stationary): lhsT = Ybuf (t,j)-chunk
            [128 (win,w), 128 (h,cs)], rhs = blkdiag(Ws_t, Ws_t)
            -> PSUM [128 (h,cs), (win, W')] -- already transposed for the h-stage
            -> Vector/Scalar drain/cast to ZT[t] bf16
  h-stage  (t-sum in PSUM): lhsT = Hblk2[t] (rows 2h+cs, cols (cs,H')),
            rhs = ZT[t] chunks, 3-matmul accumulation
            -> O [128 (cs, H'), (j8, W')] -> Vector drain f32 -> DMA out
"""

import numpy as np

B, C, H, W = 4, 48, 512, 512
T, WS = 3, 64
NCORES = 8
PH = H // 2    # 256 rows per core
NP = PH // 64  # 4 p-blocks

LAST_EXEC_NS = None
_CACHE = {}


def _build():
    import concourse.bacc as bacc
    import concourse.mybir as mybir
    from concourse.bass import MemorySpace
    from concourse.tile import TileContext

    F32 = mybir.dt.float32
    BF16 = mybir.dt.bfloat16

    nc = bacc.Bacc("TRN2", target_bir_lowering=False, debug=False, num_devices=NCORES)
    x = nc.dram_tensor("x", [C, PH, W], BF16, kind="ExternalInput")
    cstk = nc.dram_tensor("cstk", [96, 2 * T * C], BF16, kind="ExternalInput")
    wblk = nc.dram_tensor("wblk", [T, 128, 128], BF16, kind="ExternalInput")
    hblk = nc.dram_tensor("hblk", [T, 128, 128], BF16, kind="ExternalInput")
    out = nc.dram_tensor("out", [C, PH, W], BF16, kind="ExternalOutput")

    with TileContext(nc) as tc:
        with (
            tc.tile_pool(name="consts", bufs=1) as consts,
            tc.tile_pool(name="xin", bufs=2) as xin,
            tc.tile_pool(name="ybuf", bufs=2) as ypool,
            tc.tile_pool(name="ztbuf", bufs=2) as ztpool,
            tc.tile_pool(name="obuf", bufs=2) as opool,
            tc.tile_pool(name="cps", bufs=4, space=MemorySpace.PSUM) as cps,
            tc.tile_pool(name="tps", bufs=2, space=MemorySpace.PSUM) as tps,
            tc.tile_pool(name="ops", bufs=2, space=MemorySpace.PSUM) as ops,
        ):
            cstk_sb = consts.tile([96, T, 24, 2, 2], BF16)
            nc.sync.dma_start(
                out=cstk_sb,
                in_=cstk[:, :].rearrange(
                    "k (t j par s) -> k t j par s", t=T, j=24, par=2
                ),
            )
            wblk_sb = consts.tile([128, T, 128], BF16)
            nc.sync.dma_start(out=wblk_sb, in_=wblk[:, :, :].rearrange("t k m -> k t m"))
            hblk_sb = consts.tile([128, T, 128], BF16)
            nc.sync.dma_start(out=hblk_sb, in_=hblk[:, :, :].rearrange("t k m -> k t m"))

            for p in range(NP):
                # ---- load p-row: [(par2, c48), 32 hh, 512 w] (1KB runs) ----
                xt = xin.tile([96, 32, W], BF16, tag="x")
                for par in range(2):
                    nc.sync.dma_start(
                        out=xt[48 * par : 48 * par + 48],
                        in_=x[:, 64 * p + par : 64 * p + 64 : 2, :],
                    )
                ob = opool.tile([128, 3, 8, 4, 2, 64], BF16, tag="ob")
                for d in range(4):
                    wp0 = 128 * d

                    # ---- c-stage ----
                    # Ybuf [128 (win,w), (t3, j24, h64, cs2)]
                    yb = ypool.tile([128, T, 24, 64, 2], BF16, tag="y")
                    for hh in range(32):
                        cp = cps.tile([128, T, 24, 2, 2], F32, tag="c")
                        nc.tensor.matmul(
                            cp,
                            lhsT=xt[:, hh, wp0 : wp0 + 128],
                            rhs=cstk_sb,
                            start=True,
                            stop=True,
                        )
                        dst = yb[:, :, :, 2 * hh : 2 * hh + 2, :]
                        if hh % 2 == 1:
                            nc.vector.tensor_copy(out=dst, in_=cp)
                        else:
                            nc.scalar.copy(out=dst, in_=cp)

                    # ---- fused w+transpose stage per t ----
                    ztbs = []
                    for t in range(T):
                        ztb = ztpool.tile([128, 24, 128], BF16, tag=f"zt{t}")
                        for jq in range(6):
                            tp = tps.tile([128, 4, 128], F32, tag="t")
                            for i in range(4):
                                nc.tensor.matmul(
                                    tp[:, i],
                                    lhsT=yb[:, t, 4 * jq + i],
                                    rhs=wblk_sb[:, t, :],
                                    start=True,
                                    stop=True,
                                )
                            if jq % 3 != 0:
                                nc.scalar.copy(
                                    out=ztb[:, 4 * jq : 4 * jq + 4, :], in_=tp
                                )
                            else:
                                nc.vector.tensor_copy(
                                    out=ztb[:, 4 * jq : 4 * jq + 4, :], in_=tp
                                )
                        ztbs.append(ztb)

                    # ---- h-stage (t-sum in PSUM) ----
                    for win in range(2):
                        for cc in range(3):
                            op = ops.tile([128, 8, 64], F32, tag="o")
                            for t in range(T):
                                nc.tensor.matmul(
                                    op,
                                    lhsT=hblk_sb[:, t, :],
                                    rhs=ztbs[t][
                                        :,
                                        8 * cc : 8 * cc + 8,
                                        64 * win : 64 * win + 64,
                                    ],
                                    start=(t == 0),
                                    stop=(t == T - 1),
                                )
                            nc.vector.tensor_copy(
                                out=ob[:, cc, :, d, win, :], in_=op
                            )
                # ---- out DMA per p-row (1KB runs) ----
                for cs in range(2):
                    nc.sync.dma_start(
                        out=out[cs : C : 2, 64 * p : 64 * p + 64, :].rearrange(
                            "c h w -> h c w"
                        ),
                        in_=ob[64 * cs : 64 * cs + 64].rearrange(
                            "p a j d win w -> p (a j) (d win w)"
                        ),
                    )

    nc.compile()
    return nc


def _get_nc():
    if "nc" not in _CACHE:
        _CACHE["nc"] = _build()
    return _CACHE["nc"]


def _prep_consts(Ws, Hs, Cs):
    import ml_dtypes

    bf = ml_dtypes.bfloat16
    # cstk [96, (t3, j24, par2, cs2)]: rows par*48..+48 hold Cs for that parity
    cstk = np.zeros((96, T, 24, 2, 2), np.float32)
    for par in range(2):
        # cols (t, j, par, cs) = Cs[t, c, c'=2j+cs]
        cstk[48 * par : 48 * par + 48, :, :, par, :] = Cs.transpose(1, 0, 2).reshape(
            C, T, 24, 2
        )
    cstk = cstk.reshape(96, 2 * T * C)
    wblk = np.zeros((T, 128, 128), np.float32)
    hblk = np.zeros((T, 128, 128), np.float32)
    for t in range(T):
        wblk[t, 0:64, 0:64] = Ws[t]
        wblk[t, 64:128, 64:128] = Ws[t]
        # rows p = 2h+cs, cols m = cs*64+g
        for cs in range(2):
            hblk[t, cs::2, cs * 64 : cs * 64 + 64] = Hs[t]
    return cstk.astype(bf), wblk.astype(bf), hblk.astype(bf)


def kernel(x, Ws, Hs, Cs, window_size):
    global LAST_EXEC_NS
    import ml_dtypes
    from concourse.bass_utils import run_bass_kernel_spmd

    bf = ml_dtypes.bfloat16
    x = np.asarray(x, dtype=np.float32)
    Ws = np.asarray(Ws, dtype=np.float32)
    Hs = np.asarray(Hs, dtype=np.float32)
    Cs = np.asarray(Cs, dtype=np.float32)
    assert int(window_size) == WS
    assert x.shape == (B, C, H, W)

    nc = _get_nc()
    cstk, wblk, hblk = _prep_consts(Ws, Hs, Cs)
    xb = x.astype(bf)

    in_maps = []
    for core in range(NCORES):
        b, ph = core // 2, core % 2
        shard = np.ascontiguousarray(xb[b, :, ph * PH : (ph + 1) * PH, :])
        in_maps.append({"x": shard, "cstk": cstk, "wblk": wblk, "hblk": hblk})

    res = run_bass_kernel_spmd(nc, in_maps, core_ids=list(range(NCORES)))
    LAST_EXEC_NS = res.exec_time_ns

    full = np.empty((B, C, H, W), dtype=np.float32)
    for core in range(NCORES):
        b, ph = core // 2, core % 2
        full[b, :, ph * PH : (ph + 1) * PH, :] = res.results[core]["out"].astype(
            np.float32
        )
    return full


# revision 27
# speedup vs baseline: 5.4649x; 1.0108x over previous
"""FeatureProcessingBlock Trainium kernel (bf16 pipeline, v4).

out = sum_t einsum('bcphqw,twW,thH,tcC->bCpHqW', x.reshape(B,C,P,64,Q,64), Ws, Hs, Cs)

Sharding: 8 cores = (B=4) x (H-halves=2); each core gets x[b, :, ph*256:(ph+1)*256, :]
a [C=48, 256, 512] slab = 4 p-blocks x 4 double-windows (dw = two adjacent
64x64 windows in a 128-wide w-slab).

Per-core pipeline, all matmul operands bf16 (PSUM accumulates f32):
  c-stage  (data-stationary, h-pair packed): lhsT = x[(par,c)96, wp128] chunk,
            rhs = [cstk_lo | cstk_hi] [96, 288] (zero-padded halves per parity)
            -> PSUM [128 wp, (par2, t3, j24, cs2)]
            -> Scalar drain/cast to Ybuf [128 (win,w), (t3, j24, h64, cs2)]
  wT-stage (fused w-matmul + transpose; data-stationary): lhsT = Ybuf (t,j)-chunk
            [128 (win,w), 128 (h,cs)], rhs = blkdiag(Ws_t, Ws_t)
            -> PSUM [128 (h,cs), (win, W')] -- already transposed for the h-stage
            -> Vector/Scalar drain/cast to ZT[t] bf16
  h-stage  (t-sum in PSUM): lhsT = Hblk2[t] (rows 2h+cs, cols (cs,H')),
            rhs = ZT[t] chunks, 3-matmul accumulation
            -> O [128 (cs, H'), (j8, W')] -> Vector drain f32 -> DMA out
"""

import numpy as np

B, C, H, W = 4, 48, 512, 512
T, WS = 3, 64
NCORES = 8
PH = H // 2    # 256 rows per core
NP = PH // 64  # 4 p-blocks

LAST_EXEC_NS = None
_CACHE = {}


def _build():
    import concourse.bacc as bacc
    import concourse.mybir as mybir
    from concourse.bass import MemorySpace
    from concourse.tile import TileContext

    F32 = mybir.dt.float32
    BF16 = mybir.dt.bfloat16

    nc = bacc.Bacc("TRN2", target_bir_lowering=False, debug=False, num_devices=NCORES)
    x = nc.dram_tensor("x", [C, PH, W], BF16, kind="ExternalInput")
    cstk = nc.dram_tensor("cstk", [96, 2 * T * C], BF16, kind="ExternalInput")
    wblk = nc.dram_tensor("wblk", [T, 128, 128], BF16, kind="ExternalInput")
    hblk = nc.dram_tensor("hblk", [T, 128, 128], BF16, kind="ExternalInput")
    out = nc.dram_tensor("out", [C, PH, W], BF16, kind="ExternalOutput")

    with TileContext(nc) as tc:
        with (
            tc.tile_pool(name="consts", bufs=1) as consts,
            tc.tile_pool(name="xin", bufs=2) as xin,
            tc.tile_pool(name="ybuf", bufs=2) as ypool,
            tc.tile_pool(name="ztbuf", bufs=2) as ztpool,
            tc.tile_pool(name="obuf", bufs=2) as opool,
            tc.tile_pool(name="cps", bufs=4, space=MemorySpace.PSUM) as cps,
            tc.tile_pool(name="tps", bufs=2, space=MemorySpace.PSUM) as tps,
            tc.tile_pool(name="ops", bufs=2, space=MemorySpace.PSUM) as ops,
        ):
            cstk_sb = consts.tile([96, T, 24, 2, 2], BF16)
            nc.sync.dma_start(
                out=cstk_sb,
                in_=cstk[:, :].rearrange(
                    "k (t j par s) -> k t j par s", t=T, j=24, par=2
                ),
            )
            wblk_sb = consts.tile([128, T, 128], BF16)
            nc.sync.dma_start(out=wblk_sb, in_=wblk[:, :, :].rearrange("t k m -> k t m"))
            hblk_sb = consts.tile([128, T, 128], BF16)
            nc.sync.dma_start(out=hblk_sb, in_=hblk[:, :, :].rearrange("t k m -> k t m"))

            obs = {}

            def emit_c_stage(xt, yb, d):
                wp0 = 128 * d
                chunks = []
                for hh in range(32):
                    def c_chunk(hh=hh, xt=xt, yb=yb, wp0=wp0):
                        cp = cps.tile([128, T, 24, 2, 2], F32, tag="c")
                        nc.tensor.matmul(
                            cp,
                            lhsT=xt[:, hh, wp0 : wp0 + 128],
                            rhs=cstk_sb,
                            start=True,
                            stop=True,
                        )
                        dst = yb[:, :, :, 2 * hh : 2 * hh + 2, :]
                        if hh % 2 == 1:
                            nc.vector.tensor_copy(out=dst, in_=cp)
                        else:
                            nc.scalar.copy(out=dst, in_=cp)
                    chunks.append(c_chunk)
                return chunks

            def make_groups(p, d, yb):
                """wT + h + (maybe DMA) emission callbacks for one block."""
                groups = []
                ztbs = [
                    ztpool.tile([128, 24, 128], BF16, tag=f"zt{t}", name=f"ztb{t}")
                    for t in range(T)
                ]
                for t in range(T):
                    for jq in range(6):
                        def wt_group(t=t, jq=jq, yb=yb, ztb=ztbs[t]):
                            tp = tps.tile([128, 4, 128], F32, tag="t")
                            for i in range(4):
                                nc.tensor.matmul(
                                    tp[:, i],
                                    lhsT=yb[:, t, 4 * jq + i],
                                    rhs=wblk_sb[:, t, :],
                                    start=True,
                                    stop=True,
                                )
                            if jq % 3 != 0:
                                nc.scalar.copy(
                                    out=ztb[:, 4 * jq : 4 * jq + 4, :], in_=tp
                                )
                            else:
                                nc.vector.tensor_copy(
                                    out=ztb[:, 4 * jq : 4 * jq + 4, :], in_=tp
                                )
                        groups.append(wt_group)
                ob = obs[p]
                for win in range(2):
                    for cc in range(3):
                        def h_group(win=win, cc=cc, d=d, ztbs=ztbs, ob=ob):
                            op = ops.tile([128, 8, 64], F32, tag="o")
                            for t in range(T):
                                nc.tensor.matmul(
                                    op,
                                    lhsT=hblk_sb[:, t, :],
                                    rhs=ztbs[t][
                                        :,
                                        8 * cc : 8 * cc + 8,
                                        64 * win : 64 * win + 64,
                                    ],
                                    start=(t == 0),
                                    stop=(t == T - 1),
                                )
                            nc.vector.tensor_copy(
                                out=ob[:, cc, :, d, win, :], in_=op
                            )
                        groups.append(h_group)
                if d == 3:
                    def out_dma(p=p, ob=ob):
                        for cs in range(2):
                            nc.sync.dma_start(
                                out=out[
                                    cs : C : 2, 64 * p : 64 * p + 64, :
                                ].rearrange("c h w -> h c w"),
                                in_=ob[64 * cs : 64 * cs + 64].rearrange(
                                    "p a j d win w -> p (a j) (d win w)"
                                ),
                            )
                    groups.append(out_dma)
                return groups

            prev_groups = []
            xts = {}
            for k, (p, d) in enumerate([(p, d) for p in range(NP) for d in range(4)]):
                if d == 0:
                    # ---- load p-row: [(par2, c48), 32 hh, 512 w] (1KB runs) ----
                    xt = xin.tile([96, 32, W], BF16, tag="x")
                    for par in range(2):
                        nc.sync.dma_start(
                            out=xt[48 * par : 48 * par + 48],
                            in_=x[:, 64 * p + par : 64 * p + 64 : 2, :],
                        )
                    xts[p] = xt
                    obs[p] = opool.tile(
                        [128, 3, 8, 4, 2, 64], BF16, tag="ob", name=f"ob{p}"
                    )
                # Ybuf [128 (win,w), (t3, j24, h64, cs2)]
                yb = ypool.tile([128, T, 24, 64, 2], BF16, tag="y")
                chunks = emit_c_stage(xts[p], yb, d)
                # interleave this block's c-stage with previous block's wT+h
                n = max(len(chunks), len(prev_groups))
                for i in range(n):
                    if i < len(chunks):
                        chunks[i]()
                    if i < len(prev_groups):
                        prev_groups[i]()
                prev_groups = make_groups(p, d, yb)
            for g in prev_groups:
                g()

    nc.compile()
    return nc


def _get_nc():
    if "nc" not in _CACHE:
        _CACHE["nc"] = _build()
    return _CACHE["nc"]


def _prep_consts(Ws, Hs, Cs):
    import ml_dtypes

    bf = ml_dtypes.bfloat16
    # cstk [96, (t3, j24, par2, cs2)]: rows par*48..+48 hold Cs for that parity
    cstk = np.zeros((96, T, 24, 2, 2), np.float32)
    for par in range(2):
        # cols (t, j, par, cs) = Cs[t, c, c'=2j+cs]
        cstk[48 * par : 48 * par + 48, :, :, par, :] = Cs.transpose(1, 0, 2).reshape(
            C, T, 24, 2
        )
    cstk = cstk.reshape(96, 2 * T * C)
    wblk = np.zeros((T, 128, 128), np.float32)
    hblk = np.zeros((T, 128, 128), np.float32)
    for t in range(T):
        wblk[t, 0:64, 0:64] = Ws[t]
        wblk[t, 64:128, 64:128] = Ws[t]
        # rows p = 2h+cs, cols m = cs*64+g
        for cs in range(2):
            hblk[t, cs::2, cs * 64 : cs * 64 + 64] = Hs[t]
    return cstk.astype(bf), wblk.astype(bf), hblk.astype(bf)


def kernel(x, Ws, Hs, Cs, window_size):
    global LAST_EXEC_NS
    import ml_dtypes
    from concourse.bass_utils import run_bass_kernel_spmd

    bf = ml_dtypes.bfloat16
    x = np.asarray(x, dtype=np.float32)
    Ws = np.asarray(Ws, dtype=np.float32)
    Hs = np.asarray(Hs, dtype=np.float32)
    Cs = np.asarray(Cs, dtype=np.float32)
    assert int(window_size) == WS
    assert x.shape == (B, C, H, W)

    nc = _get_nc()
    cstk, wblk, hblk = _prep_consts(Ws, Hs, Cs)
    xb = x.astype(bf)

    in_maps = []
    for core in range(NCORES):
        b, ph = core // 2, core % 2
        shard = np.ascontiguousarray(xb[b, :, ph * PH : (ph + 1) * PH, :])
        in_maps.append({"x": shard, "cstk": cstk, "wblk": wblk, "hblk": hblk})

    res = run_bass_kernel_spmd(nc, in_maps, core_ids=list(range(NCORES)))
    LAST_EXEC_NS = res.exec_time_ns

    full = np.empty((B, C, H, W), dtype=np.float32)
    for core in range(NCORES):
        b, ph = core // 2, core % 2
        full[b, :, ph * PH : (ph + 1) * PH, :] = res.results[core]["out"].astype(
            np.float32
        )
    return full


# revision 31
# speedup vs baseline: 5.7615x; 1.0543x over previous
"""FeatureProcessingBlock Trainium kernel (bf16 pipeline, v4).

out = sum_t einsum('bcphqw,twW,thH,tcC->bCpHqW', x.reshape(B,C,P,64,Q,64), Ws, Hs, Cs)

Sharding: 8 cores = (B=4) x (H-halves=2); each core gets x[b, :, ph*256:(ph+1)*256, :]
a [C=48, 256, 512] slab = 4 p-blocks x 4 double-windows (dw = two adjacent
64x64 windows in a 128-wide w-slab).

Per-core pipeline, all matmul operands bf16 (PSUM accumulates f32):
  c-stage  (data-stationary, h-pair packed): lhsT = x[(par,c)96, wp128] chunk,
            rhs = [cstk_lo | cstk_hi] [96, 288] (zero-padded halves per parity)
            -> PSUM [128 wp, (par2, t3, j24, cs2)]
            -> Scalar drain/cast to Ybuf [128 (win,w), (t3, j24, h64, cs2)]
  wT-stage (fused w-matmul + transpose; data-stationary): lhsT = Ybuf (t,j)-chunk
            [128 (win,w), 128 (h,cs)], rhs = blkdiag(Ws_t, Ws_t)
            -> PSUM [128 (h,cs), (win, W')] -- already transposed for the h-stage
            -> Vector/Scalar drain/cast to ZT[t] bf16
  h-stage  (t-sum in PSUM): lhsT = Hblk2[t] (rows 2h+cs, cols (cs,H')),
            rhs = ZT[t] chunks, 3-matmul accumulation
            -> O [128 (cs, H'), (j8, W')] -> Vector drain f32 -> DMA out
"""

import numpy as np

B, C, H, W = 4, 48, 512, 512
T, WS = 3, 64
NCORES = 8
PH = H // 2    # 256 rows per core
NP = PH // 64  # 4 p-blocks

LAST_EXEC_NS = None
_CACHE = {}


def _build():
    import concourse.bacc as bacc
    import concourse.mybir as mybir
    from concourse.bass import MemorySpace
    from concourse.tile import TileContext

    F32 = mybir.dt.float32
    BF16 = mybir.dt.bfloat16

    nc = bacc.Bacc("TRN2", target_bir_lowering=False, debug=False, num_devices=NCORES)
    x = nc.dram_tensor("x", [C, PH, W], BF16, kind="ExternalInput")
    cstk = nc.dram_tensor("cstk", [96, 2 * T * C], BF16, kind="ExternalInput")
    wblk = nc.dram_tensor("wblk", [T, 128, 128], BF16, kind="ExternalInput")
    hblk = nc.dram_tensor("hblk", [T, 128, 128], BF16, kind="ExternalInput")
    out = nc.dram_tensor("out", [C, PH, W], BF16, kind="ExternalOutput")

    with TileContext(nc) as tc:
        with (
            tc.tile_pool(name="consts", bufs=1) as consts,
            tc.tile_pool(name="xin", bufs=3) as xin,
            tc.tile_pool(name="ybuf", bufs=2) as ypool,
            tc.tile_pool(name="ztbuf", bufs=2) as ztpool,
            tc.tile_pool(name="obuf", bufs=2) as opool,
            tc.tile_pool(name="cps", bufs=4, space=MemorySpace.PSUM) as cps,
            tc.tile_pool(name="tps", bufs=2, space=MemorySpace.PSUM) as tps,
            tc.tile_pool(name="ops", bufs=2, space=MemorySpace.PSUM) as ops,
        ):
            cstk_sb = consts.tile([96, T, 24, 2, 2], BF16)
            nc.sync.dma_start(
                out=cstk_sb,
                in_=cstk[:, :].rearrange(
                    "k (t j par s) -> k t j par s", t=T, j=24, par=2
                ),
            )
            wblk_sb = consts.tile([128, T, 128], BF16)
            nc.sync.dma_start(out=wblk_sb, in_=wblk[:, :, :].rearrange("t k m -> k t m"))
            hblk_sb = consts.tile([128, T, 128], BF16)
            nc.sync.dma_start(out=hblk_sb, in_=hblk[:, :, :].rearrange("t k m -> k t m"))

            obs = {}

            def emit_c_stage(xt, yb, d):
                wp0 = 128 * (d % 2)
                chunks = []
                for hh in range(32):
                    def c_chunk(hh=hh, xt=xt, yb=yb, wp0=wp0):
                        cp = cps.tile([128, T, 24, 2, 2], F32, tag="c")
                        nc.tensor.matmul(
                            cp,
                            lhsT=xt[:, hh, wp0 : wp0 + 128],
                            rhs=cstk_sb,
                            start=True,
                            stop=True,
                        )
                        dst = yb[:, :, :, 2 * hh : 2 * hh + 2, :]
                        if hh % 2 == 1:
                            nc.vector.tensor_copy(out=dst, in_=cp)
                        else:
                            nc.scalar.copy(out=dst, in_=cp)
                    chunks.append(c_chunk)
                return chunks

            def make_groups(p, d, yb):
                """wT + h + (maybe DMA) emission callbacks for one block."""
                groups = []
                ztbs = [
                    ztpool.tile([128, 24, 128], BF16, tag=f"zt{t}", name=f"ztb{t}")
                    for t in range(T)
                ]
                for t in range(T):
                    for jq in range(6):
                        def wt_group(t=t, jq=jq, yb=yb, ztb=ztbs[t]):
                            tp = tps.tile([128, 4, 128], F32, tag="t")
                            for i in range(4):
                                nc.tensor.matmul(
                                    tp[:, i],
                                    lhsT=yb[:, t, 4 * jq + i],
                                    rhs=wblk_sb[:, t, :],
                                    start=True,
                                    stop=True,
                                )
                            if jq % 3 != 0:
                                nc.scalar.copy(
                                    out=ztb[:, 4 * jq : 4 * jq + 4, :], in_=tp
                                )
                            else:
                                nc.vector.tensor_copy(
                                    out=ztb[:, 4 * jq : 4 * jq + 4, :], in_=tp
                                )
                        groups.append(wt_group)
                ob = obs[p]
                for win in range(2):
                    for cc in range(3):
                        def h_group(win=win, cc=cc, d=d, ztbs=ztbs, ob=ob):
                            op = ops.tile([128, 8, 64], F32, tag="o")
                            for t in range(T):
                                nc.tensor.matmul(
                                    op,
                                    lhsT=hblk_sb[:, t, :],
                                    rhs=ztbs[t][
                                        :,
                                        8 * cc : 8 * cc + 8,
                                        64 * win : 64 * win + 64,
                                    ],
                                    start=(t == 0),
                                    stop=(t == T - 1),
                                )
                            nc.vector.tensor_copy(
                                out=ob[:, cc, :, d, win, :], in_=op
                            )
                        groups.append(h_group)
                if d % 2 == 1:
                    def out_dma(p=p, d=d, ob=ob):
                        for cs in range(2):
                            nc.sync.dma_start(
                                out=out[
                                    cs : C : 2,
                                    64 * p : 64 * p + 64,
                                    256 * (d // 2) : 256 * (d // 2) + 256,
                                ].rearrange("c h w -> h c w"),
                                in_=ob[
                                    64 * cs : 64 * cs + 64, :, :, d - 1 : d + 1
                                ].rearrange("p a j d win w -> p (a j) (d win w)"),
                            )
                    groups.append(out_dma)
                return groups

            prev_groups = []
            xts = {}
            for k, (p, d) in enumerate([(p, d) for p in range(NP) for d in range(4)]):
                if d % 2 == 0:
                    # ---- load half p-row: [(par2, c48), 32 hh, 256 w] (512B runs)
                    xt = xin.tile([96, 32, 256], BF16, tag="x", name=f"x{k}")
                    for par in range(2):
                        nc.sync.dma_start(
                            out=xt[48 * par : 48 * par + 48],
                            in_=x[
                                :,
                                64 * p + par : 64 * p + 64 : 2,
                                256 * (d // 2) : 256 * (d // 2) + 256,
                            ],
                        )
                    xts[p] = xt
                if d == 0:
                    obs[p] = opool.tile(
                        [128, 3, 8, 4, 2, 64], BF16, tag="ob", name=f"ob{p}"
                    )
                # Ybuf [128 (win,w), (t3, j24, h64, cs2)]
                yb = ypool.tile([128, T, 24, 64, 2], BF16, tag="y")
                chunks = emit_c_stage(xts[p], yb, d)
                # interleave this block's c-stage with previous block's wT+h
                n = max(len(chunks), len(prev_groups))
                for i in range(n):
                    if i < len(chunks):
                        chunks[i]()
                    if i < len(prev_groups):
                        prev_groups[i]()
                prev_groups = make_groups(p, d, yb)
            for g in prev_groups:
                g()

    nc.compile()
    return nc


def _get_nc():
    if "nc" not in _CACHE:
        _CACHE["nc"] = _build()
    return _CACHE["nc"]


def _prep_consts(Ws, Hs, Cs):
    import ml_dtypes

    bf = ml_dtypes.bfloat16
    # cstk [96, (t3, j24, par2, cs2)]: rows par*48..+48 hold Cs for that parity
    cstk = np.zeros((96, T, 24, 2, 2), np.float32)
    for par in range(2):
        # cols (t, j, par, cs) = Cs[t, c, c'=2j+cs]
        cstk[48 * par : 48 * par + 48, :, :, par, :] = Cs.transpose(1, 0, 2).reshape(
            C, T, 24, 2
        )
    cstk = cstk.reshape(96, 2 * T * C)
    wblk = np.zeros((T, 128, 128), np.float32)
    hblk = np.zeros((T, 128, 128), np.float32)
    for t in range(T):
        wblk[t, 0:64, 0:64] = Ws[t]
        wblk[t, 64:128, 64:128] = Ws[t]
        # rows p = 2h+cs, cols m = cs*64+g
        for cs in range(2):
            hblk[t, cs::2, cs * 64 : cs * 64 + 64] = Hs[t]
    return cstk.astype(bf), wblk.astype(bf), hblk.astype(bf)


def kernel(x, Ws, Hs, Cs, window_size):
    global LAST_EXEC_NS
    import ml_dtypes
    from concourse.bass_utils import run_bass_kernel_spmd

    bf = ml_dtypes.bfloat16
    x = np.asarray(x, dtype=np.float32)
    Ws = np.asarray(Ws, dtype=np.float32)
    Hs = np.asarray(Hs, dtype=np.float32)
    Cs = np.asarray(Cs, dtype=np.float32)
    assert int(window_size) == WS
    assert x.shape == (B, C, H, W)

    nc = _get_nc()
    cstk, wblk, hblk = _prep_consts(Ws, Hs, Cs)
    xb = x.astype(bf)

    in_maps = []
    for core in range(NCORES):
        b, ph = core // 2, core % 2
        shard = np.ascontiguousarray(xb[b, :, ph * PH : (ph + 1) * PH, :])
        in_maps.append({"x": shard, "cstk": cstk, "wblk": wblk, "hblk": hblk})

    res = run_bass_kernel_spmd(nc, in_maps, core_ids=list(range(NCORES)))
    LAST_EXEC_NS = res.exec_time_ns

    full = np.empty((B, C, H, W), dtype=np.float32)
    for core in range(NCORES):
        b, ph = core // 2, core % 2
        full[b, :, ph * PH : (ph + 1) * PH, :] = res.results[core]["out"].astype(
            np.float32
        )
    return full


# revision 34
# speedup vs baseline: 6.0932x; 1.0576x over previous
"""FeatureProcessingBlock Trainium kernel (bf16 pipeline, v4).

out = sum_t einsum('bcphqw,twW,thH,tcC->bCpHqW', x.reshape(B,C,P,64,Q,64), Ws, Hs, Cs)

Sharding: 8 cores = (B=4) x (H-halves=2); each core gets x[b, :, ph*256:(ph+1)*256, :]
a [C=48, 256, 512] slab = 4 p-blocks x 4 double-windows (dw = two adjacent
64x64 windows in a 128-wide w-slab).

Per-core pipeline, all matmul operands bf16 (PSUM accumulates f32):
  c-stage  (data-stationary, h-pair packed): lhsT = x[(par,c)96, wp128] chunk,
            rhs = [cstk_lo | cstk_hi] [96, 288] (zero-padded halves per parity)
            -> PSUM [128 wp, (par2, t3, j24, cs2)]
            -> Scalar drain/cast to Ybuf [128 (win,w), (t3, j24, h64, cs2)]
  wT-stage (fused w-matmul + transpose; data-stationary): lhsT = Ybuf (t,j)-chunk
            [128 (win,w), 128 (h,cs)], rhs = blkdiag(Ws_t, Ws_t)
            -> PSUM [128 (h,cs), (win, W')] -- already transposed for the h-stage
            -> Vector/Scalar drain/cast to ZT[t] bf16
  h-stage  (t-sum in PSUM): lhsT = Hblk2[t] (rows 2h+cs, cols (cs,H')),
            rhs = ZT[t] chunks, 3-matmul accumulation
            -> O [128 (cs, H'), (j8, W')] -> Vector drain f32 -> DMA out
"""

import numpy as np

B, C, H, W = 4, 48, 512, 512
T, WS = 3, 64
NCORES = 8
PH = H // 2    # 256 rows per core
NP = PH // 64  # 4 p-blocks

LAST_EXEC_NS = None
_CACHE = {}


def _build():
    import concourse.bacc as bacc
    import concourse.mybir as mybir
    from concourse.bass import MemorySpace
    from concourse.tile import TileContext

    F32 = mybir.dt.float32
    BF16 = mybir.dt.bfloat16

    nc = bacc.Bacc("TRN2", target_bir_lowering=False, debug=False, num_devices=NCORES)
    x = nc.dram_tensor("x", [C, PH, W], BF16, kind="ExternalInput")
    cstk = nc.dram_tensor("cstk", [96, 2 * T * C], BF16, kind="ExternalInput")
    wblk = nc.dram_tensor("wblk", [T, 128, 128], BF16, kind="ExternalInput")
    hblk = nc.dram_tensor("hblk", [T, 128, 128], BF16, kind="ExternalInput")
    out = nc.dram_tensor("out", [C, PH, W], BF16, kind="ExternalOutput")

    with TileContext(nc) as tc:
        with (
            tc.tile_pool(name="consts", bufs=1) as consts,
            tc.tile_pool(name="xin", bufs=3) as xin,
            tc.tile_pool(name="ybuf", bufs=2) as ypool,
            tc.tile_pool(name="ztbuf", bufs=2) as ztpool,
            tc.tile_pool(name="obuf", bufs=2) as opool,
            tc.tile_pool(name="cps", bufs=4, space=MemorySpace.PSUM) as cps,
            tc.tile_pool(name="tps", bufs=2, space=MemorySpace.PSUM) as tps,
            tc.tile_pool(name="ops", bufs=2, space=MemorySpace.PSUM) as ops,
        ):
            cstk_sb = consts.tile([96, T, 24, 2, 2], BF16)
            nc.sync.dma_start(
                out=cstk_sb,
                in_=cstk[:, :].rearrange(
                    "k (t j par s) -> k t j par s", t=T, j=24, par=2
                ),
            )
            wblk_sb = consts.tile([128, T, 128], BF16)
            nc.sync.dma_start(out=wblk_sb, in_=wblk[:, :, :].rearrange("t k m -> k t m"))
            hblk_sb = consts.tile([128, T, 128], BF16)
            nc.sync.dma_start(out=hblk_sb, in_=hblk[:, :, :].rearrange("t k m -> k t m"))

            obs = {}

            def emit_c_stage(xt, yb, d):
                wp0 = 128 * (d % 2)
                chunks = []
                for hh in range(32):
                    def c_chunk(hh=hh, xt=xt, yb=yb, wp0=wp0):
                        cp = cps.tile([128, T, 24, 2, 2], F32, tag="c")
                        nc.tensor.matmul(
                            cp,
                            lhsT=xt[hh // 8][:, hh % 8, wp0 : wp0 + 128],
                            rhs=cstk_sb,
                            start=True,
                            stop=True,
                        )
                        dst = yb[:, :, :, 2 * hh : 2 * hh + 2, :]
                        if hh % 2 == 1:
                            nc.vector.tensor_copy(out=dst, in_=cp)
                        else:
                            nc.scalar.copy(out=dst, in_=cp)
                    chunks.append(c_chunk)
                return chunks

            def make_groups(p, d, yb):
                """wT + h + (maybe DMA) emission callbacks for one block."""
                groups = []
                ztbs = [
                    ztpool.tile([128, 24, 128], BF16, tag=f"zt{t}", name=f"ztb{t}")
                    for t in range(T)
                ]
                for t in range(T):
                    for jq in range(6):
                        def wt_group(t=t, jq=jq, yb=yb, ztb=ztbs[t]):
                            tp = tps.tile([128, 4, 128], F32, tag="t")
                            for i in range(4):
                                nc.tensor.matmul(
                                    tp[:, i],
                                    lhsT=yb[:, t, 4 * jq + i],
                                    rhs=wblk_sb[:, t, :],
                                    start=True,
                                    stop=True,
                                )
                            if jq % 3 != 0:
                                nc.scalar.copy(
                                    out=ztb[:, 4 * jq : 4 * jq + 4, :], in_=tp
                                )
                            else:
                                nc.vector.tensor_copy(
                                    out=ztb[:, 4 * jq : 4 * jq + 4, :], in_=tp
                                )
                        groups.append(wt_group)
                ob = obs[p]
                for win in range(2):
                    for cc in range(3):
                        def h_group(win=win, cc=cc, d=d, ztbs=ztbs, ob=ob):
                            op = ops.tile([128, 8, 64], F32, tag="o")
                            for t in range(T):
                                nc.tensor.matmul(
                                    op,
                                    lhsT=hblk_sb[:, t, :],
                                    rhs=ztbs[t][
                                        :,
                                        8 * cc : 8 * cc + 8,
                                        64 * win : 64 * win + 64,
                                    ],
                                    start=(t == 0),
                                    stop=(t == T - 1),
                                )
                            nc.vector.tensor_copy(
                                out=ob[:, cc, :, d, win, :], in_=op
                            )
                        groups.append(h_group)
                def out_dma(p=p, d=d, ob=ob):
                    for cs in range(2):
                        for a in range(3):
                            nc.sync.dma_start(
                                out=out[
                                    16 * a + cs : 16 * a + 16 : 2,
                                    64 * p : 64 * p + 64,
                                    128 * d : 128 * d + 128,
                                ].rearrange("c h w -> h c w"),
                                in_=ob[
                                    64 * cs : 64 * cs + 64, a, :, d
                                ].rearrange("p j win w -> p j (win w)"),
                            )
                groups.append(out_dma)
                return groups

            prev_groups = []
            xts = {}
            for k, (p, d) in enumerate([(p, d) for p in range(NP) for d in range(4)]):
                if d % 2 == 0:
                    # ---- load half p-row in 4 hh-pieces: [(par2, c48), 8 hh, 256 w]
                    pieces = []
                    for q in range(4):
                        xq = xin.tile([96, 8, 256], BF16, tag=f"x{q}", name=f"x{k}_{q}")
                        for par in range(2):
                            nc.sync.dma_start(
                                out=xq[48 * par : 48 * par + 48],
                                in_=x[
                                    :,
                                    64 * p + 16 * q + par : 64 * p + 16 * q + 16 : 2,
                                    256 * (d // 2) : 256 * (d // 2) + 256,
                                ],
                            )
                        pieces.append(xq)
                    xts[p] = pieces
                if d == 0:
                    obs[p] = opool.tile(
                        [128, 3, 8, 4, 2, 64], BF16, tag="ob", name=f"ob{p}"
                    )
                # Ybuf [128 (win,w), (t3, j24, h64, cs2)]
                yb = ypool.tile([128, T, 24, 64, 2], BF16, tag="y")
                chunks = emit_c_stage(xts[p], yb, d)
                # interleave this block's c-stage with previous block's wT+h
                n = max(len(chunks), len(prev_groups))
                for i in range(n):
                    if i < len(chunks):
                        chunks[i]()
                    if i < len(prev_groups):
                        prev_groups[i]()
                prev_groups = make_groups(p, d, yb)
            for g in prev_groups:
                g()

    nc.compile()
    return nc


def _get_nc():
    if "nc" not in _CACHE:
        _CACHE["nc"] = _build()
    return _CACHE["nc"]


def _prep_consts(Ws, Hs, Cs):
    import ml_dtypes

    bf = ml_dtypes.bfloat16
    # cstk [96, (t3, j24, par2, cs2)]: rows par*48..+48 hold Cs for that parity
    cstk = np.zeros((96, T, 24, 2, 2), np.float32)
    for par in range(2):
        # cols (t, j, par, cs) = Cs[t, c, c'=2j+cs]
        cstk[48 * par : 48 * par + 48, :, :, par, :] = Cs.transpose(1, 0, 2).reshape(
            C, T, 24, 2
        )
    cstk = cstk.reshape(96, 2 * T * C)
    wblk = np.zeros((T, 128, 128), np.float32)
    hblk = np.zeros((T, 128, 128), np.float32)
    for t in range(T):
        wblk[t, 0:64, 0:64] = Ws[t]
        wblk[t, 64:128, 64:128] = Ws[t]
        # rows p = 2h+cs, cols m = cs*64+g
        for cs in range(2):
            hblk[t, cs::2, cs * 64 : cs * 64 + 64] = Hs[t]
    return cstk.astype(bf), wblk.astype(bf), hblk.astype(bf)


def kernel(x, Ws, Hs, Cs, window_size):
    global LAST_EXEC_NS
    import ml_dtypes
    from concourse.bass_utils import run_bass_kernel_spmd

    bf = ml_dtypes.bfloat16
    x = np.asarray(x, dtype=np.float32)
    Ws = np.asarray(Ws, dtype=np.float32)
    Hs = np.asarray(Hs, dtype=np.float32)
    Cs = np.asarray(Cs, dtype=np.float32)
    assert int(window_size) == WS
    assert x.shape == (B, C, H, W)

    nc = _get_nc()
    cstk, wblk, hblk = _prep_consts(Ws, Hs, Cs)
    xb = x.astype(bf)

    in_maps = []
    for core in range(NCORES):
        b, ph = core // 2, core % 2
        shard = np.ascontiguousarray(xb[b, :, ph * PH : (ph + 1) * PH, :])
        in_maps.append({"x": shard, "cstk": cstk, "wblk": wblk, "hblk": hblk})

    res = run_bass_kernel_spmd(nc, in_maps, core_ids=list(range(NCORES)))
    LAST_EXEC_NS = res.exec_time_ns

    full = np.empty((B, C, H, W), dtype=np.float32)
    for core in range(NCORES):
        b, ph = core // 2, core % 2
        full[b, :, ph * PH : (ph + 1) * PH, :] = res.results[core]["out"].astype(
            np.float32
        )
    return full


# revision 36
# speedup vs baseline: 6.1208x; 1.0045x over previous
"""FeatureProcessingBlock Trainium kernel (bf16 pipeline, v4).

out = sum_t einsum('bcphqw,twW,thH,tcC->bCpHqW', x.reshape(B,C,P,64,Q,64), Ws, Hs, Cs)

Sharding: 8 cores = (B=4) x (H-halves=2); each core gets x[b, :, ph*256:(ph+1)*256, :]
a [C=48, 256, 512] slab = 4 p-blocks x 4 double-windows (dw = two adjacent
64x64 windows in a 128-wide w-slab).

Per-core pipeline, all matmul operands bf16 (PSUM accumulates f32):
  c-stage  (data-stationary, h-pair packed): lhsT = x[(par,c)96, wp128] chunk,
            rhs = [cstk_lo | cstk_hi] [96, 288] (zero-padded halves per parity)
            -> PSUM [128 wp, (par2, t3, j24, cs2)]
            -> Scalar drain/cast to Ybuf [128 (win,w), (t3, j24, h64, cs2)]
  wT-stage (fused w-matmul + transpose; data-stationary): lhsT = Ybuf (t,j)-chunk
            [128 (win,w), 128 (h,cs)], rhs = blkdiag(Ws_t, Ws_t)
            -> PSUM [128 (h,cs), (win, W')] -- already transposed for the h-stage
            -> Vector/Scalar drain/cast to ZT[t] bf16
  h-stage  (t-sum in PSUM): lhsT = Hblk2[t] (rows 2h+cs, cols (cs,H')),
            rhs = ZT[t] chunks, 3-matmul accumulation
            -> O [128 (cs, H'), (j8, W')] -> Vector drain f32 -> DMA out
"""

import numpy as np

B, C, H, W = 4, 48, 512, 512
T, WS = 3, 64
NCORES = 8
PH = H // 2    # 256 rows per core
NP = PH // 64  # 4 p-blocks

LAST_EXEC_NS = None
_CACHE = {}


def _build():
    import concourse.bacc as bacc
    import concourse.mybir as mybir
    from concourse.bass import MemorySpace
    from concourse.tile import TileContext

    F32 = mybir.dt.float32
    BF16 = mybir.dt.bfloat16

    nc = bacc.Bacc("TRN2", target_bir_lowering=False, debug=False, num_devices=NCORES)
    x = nc.dram_tensor("x", [C, PH, W], BF16, kind="ExternalInput")
    cstk = nc.dram_tensor("cstk", [96, 2 * T * C], BF16, kind="ExternalInput")
    wblk = nc.dram_tensor("wblk", [T, 128, 128], BF16, kind="ExternalInput")
    hblk = nc.dram_tensor("hblk", [T, 128, 128], BF16, kind="ExternalInput")
    out = nc.dram_tensor("out", [C, PH, W], BF16, kind="ExternalOutput")

    with TileContext(nc) as tc:
        with (
            tc.tile_pool(name="consts", bufs=1) as consts,
            tc.tile_pool(name="xin", bufs=3) as xin,
            tc.tile_pool(name="ybuf", bufs=2) as ypool,
            tc.tile_pool(name="ztbuf", bufs=2) as ztpool,
            tc.tile_pool(name="obuf", bufs=2) as opool,
            tc.tile_pool(name="cps", bufs=4, space=MemorySpace.PSUM) as cps,
            tc.tile_pool(name="tps", bufs=2, space=MemorySpace.PSUM) as tps,
            tc.tile_pool(name="ops", bufs=2, space=MemorySpace.PSUM) as ops,
        ):
            cstk_sb = consts.tile([96, T, 24, 2, 2], BF16)
            nc.sync.dma_start(
                out=cstk_sb,
                in_=cstk[:, :].rearrange(
                    "k (t j par s) -> k t j par s", t=T, j=24, par=2
                ),
            )
            wblk_sb = consts.tile([128, T, 128], BF16)
            hblk_sb = consts.tile([128, T, 128], BF16)
            wh_loaded = []

            def load_wh():
                nc.sync.dma_start(
                    out=wblk_sb, in_=wblk[:, :, :].rearrange("t k m -> k t m")
                )
                nc.sync.dma_start(
                    out=hblk_sb, in_=hblk[:, :, :].rearrange("t k m -> k t m")
                )
                wh_loaded.append(True)

            obs = {}

            def emit_c_stage(xt, yb, d):
                wp0 = 128 * (d % 2)
                chunks = []
                for hh in range(32):
                    def c_chunk(hh=hh, xt=xt, yb=yb, wp0=wp0):
                        cp = cps.tile([128, T, 24, 2, 2], F32, tag="c")
                        nc.tensor.matmul(
                            cp,
                            lhsT=xt[hh // 8][:, hh % 8, wp0 : wp0 + 128],
                            rhs=cstk_sb,
                            start=True,
                            stop=True,
                        )
                        dst = yb[:, :, :, 2 * hh : 2 * hh + 2, :]
                        if hh % 2 == 1:
                            nc.vector.tensor_copy(out=dst, in_=cp)
                        else:
                            nc.scalar.copy(out=dst, in_=cp)
                    chunks.append(c_chunk)
                return chunks

            def make_groups(p, d, yb):
                """wT + h + (maybe DMA) emission callbacks for one block."""
                groups = []
                ztbs = [
                    ztpool.tile([128, 24, 128], BF16, tag=f"zt{t}", name=f"ztb{t}")
                    for t in range(T)
                ]
                for t in range(T):
                    for jq in range(6):
                        def wt_group(t=t, jq=jq, yb=yb, ztb=ztbs[t]):
                            tp = tps.tile([128, 4, 128], F32, tag="t")
                            for i in range(4):
                                nc.tensor.matmul(
                                    tp[:, i],
                                    lhsT=yb[:, t, 4 * jq + i],
                                    rhs=wblk_sb[:, t, :],
                                    start=True,
                                    stop=True,
                                )
                            if jq % 3 != 0:
                                nc.scalar.copy(
                                    out=ztb[:, 4 * jq : 4 * jq + 4, :], in_=tp
                                )
                            else:
                                nc.vector.tensor_copy(
                                    out=ztb[:, 4 * jq : 4 * jq + 4, :], in_=tp
                                )
                        groups.append(wt_group)
                ob = obs[p]
                for win in range(2):
                    for cc in range(3):
                        def h_group(win=win, cc=cc, d=d, ztbs=ztbs, ob=ob):
                            op = ops.tile([128, 8, 64], F32, tag="o")
                            for t in range(T):
                                nc.tensor.matmul(
                                    op,
                                    lhsT=hblk_sb[:, t, :],
                                    rhs=ztbs[t][
                                        :,
                                        8 * cc : 8 * cc + 8,
                                        64 * win : 64 * win + 64,
                                    ],
                                    start=(t == 0),
                                    stop=(t == T - 1),
                                )
                            nc.vector.tensor_copy(
                                out=ob[:, cc, :, d, win, :], in_=op
                            )
                        groups.append(h_group)
                def out_dma(p=p, d=d, ob=ob):
                    for cs in range(2):
                        for a in range(3):
                            nc.sync.dma_start(
                                out=out[
                                    16 * a + cs : 16 * a + 16 : 2,
                                    64 * p : 64 * p + 64,
                                    128 * d : 128 * d + 128,
                                ].rearrange("c h w -> h c w"),
                                in_=ob[
                                    64 * cs : 64 * cs + 64, a, :, d
                                ].rearrange("p j win w -> p j (win w)"),
                            )
                groups.append(out_dma)
                return groups

            prev_groups = []
            xts = {}
            for k, (p, d) in enumerate([(p, d) for p in range(NP) for d in range(4)]):
                if d % 2 == 0:
                    # ---- load half p-row in 4 hh-pieces: [(par2, c48), 8 hh, 256 w]
                    pieces = []
                    for q in range(4):
                        xq = xin.tile([96, 8, 256], BF16, tag=f"x{q}", name=f"x{k}_{q}")
                        for par in range(2):
                            nc.sync.dma_start(
                                out=xq[48 * par : 48 * par + 48],
                                in_=x[
                                    :,
                                    64 * p + 16 * q + par : 64 * p + 16 * q + 16 : 2,
                                    256 * (d // 2) : 256 * (d // 2) + 256,
                                ],
                            )
                        pieces.append(xq)
                    xts[p] = pieces
                    if not wh_loaded:
                        load_wh()
                if d == 0:
                    obs[p] = opool.tile(
                        [128, 3, 8, 4, 2, 64], BF16, tag="ob", name=f"ob{p}"
                    )
                # Ybuf [128 (win,w), (t3, j24, h64, cs2)]
                yb = ypool.tile([128, T, 24, 64, 2], BF16, tag="y")
                chunks = emit_c_stage(xts[p], yb, d)
                # interleave this block's c-stage with previous block's wT+h
                n = max(len(chunks), len(prev_groups))
                for i in range(n):
                    if i < len(chunks):
                        chunks[i]()
                    if i < len(prev_groups):
                        prev_groups[i]()
                prev_groups = make_groups(p, d, yb)
            for g in prev_groups:
                g()

    nc.compile()
    return nc


def _get_nc():
    if "nc" not in _CACHE:
        _CACHE["nc"] = _build()
    return _CACHE["nc"]


def _prep_consts(Ws, Hs, Cs):
    import ml_dtypes

    bf = ml_dtypes.bfloat16
    # cstk [96, (t3, j24, par2, cs2)]: rows par*48..+48 hold Cs for that parity
    cstk = np.zeros((96, T, 24, 2, 2), np.float32)
    for par in range(2):
        # cols (t, j, par, cs) = Cs[t, c, c'=2j+cs]
        cstk[48 * par : 48 * par + 48, :, :, par, :] = Cs.transpose(1, 0, 2).reshape(
            C, T, 24, 2
        )
    cstk = cstk.reshape(96, 2 * T * C)
    wblk = np.zeros((T, 128, 128), np.float32)
    hblk = np.zeros((T, 128, 128), np.float32)
    for t in range(T):
        wblk[t, 0:64, 0:64] = Ws[t]
        wblk[t, 64:128, 64:128] = Ws[t]
        # rows p = 2h+cs, cols m = cs*64+g
        for cs in range(2):
            hblk[t, cs::2, cs * 64 : cs * 64 + 64] = Hs[t]
    return cstk.astype(bf), wblk.astype(bf), hblk.astype(bf)


def kernel(x, Ws, Hs, Cs, window_size):
    global LAST_EXEC_NS
    import ml_dtypes
    from concourse.bass_utils import run_bass_kernel_spmd

    bf = ml_dtypes.bfloat16
    x = np.asarray(x, dtype=np.float32)
    Ws = np.asarray(Ws, dtype=np.float32)
    Hs = np.asarray(Hs, dtype=np.float32)
    Cs = np.asarray(Cs, dtype=np.float32)
    assert int(window_size) == WS
    assert x.shape == (B, C, H, W)

    nc = _get_nc()
    cstk, wblk, hblk = _prep_consts(Ws, Hs, Cs)
    xb = x.astype(bf)

    in_maps = []
    for core in range(NCORES):
        b, ph = core // 2, core % 2
        shard = np.ascontiguousarray(xb[b, :, ph * PH : (ph + 1) * PH, :])
        in_maps.append({"x": shard, "cstk": cstk, "wblk": wblk, "hblk": hblk})

    res = run_bass_kernel_spmd(nc, in_maps, core_ids=list(range(NCORES)))
    LAST_EXEC_NS = res.exec_time_ns

    full = np.empty((B, C, H, W), dtype=np.float32)
    for core in range(NCORES):
        b, ph = core // 2, core % 2
        full[b, :, ph * PH : (ph + 1) * PH, :] = res.results[core]["out"].astype(
            np.float32
        )
    return full
